# revision 2
# baseline (speedup 1.0000x reference)
"""MultiHeadDiffAttention Trainium2 kernel, v3.

Strategy (8 NeuronCores, SPMD), same sharding as baseline:
  core c handles b = c//4, heads 4*(c%4)..4*(c%4)+3.

Perf structure (CoreSim-modeled, HW-verified numerics):
  1. QKV projection runs as error-compensated split-fp8 (e4m3 hi+lo with a
     shared power-of-2 scale, 3 of 4 cross terms) using DoubleRow matmuls:
     each instruction contracts 256 rows at 0.5 cycles/row, so the
     projection costs 0.75x the fp32r schedule at ~bf16 accuracy.
     x and W are split/interleaved host-side; all descale factors fold into
     the exp() scale and the (host-prepared) W_proj.
  2. The softmax denominator comes from a bf16 DVE accumulation of the exp
     tiles plus one Pool partition_all_reduce - no PE ones-matmuls.
  3. exp() runs 1024-wide (2 PSUM banks per logits tile) to halve ACT's
     per-instruction access overhead; q/k are stored fp16 (smaller scales)
     so the large-logit heads keep ~1e-3 score accuracy.
  4. Software pipelining: the t-block-1 Q projection is injected 3 matmuls
     per s-iteration into the first attention block (which is otherwise
     ACT-bound), and the output projection of block 0 is interleaved with
     the second attention block, so PE never drains between phases.
"""

import math

import numpy as np

B, T, E = 2, 2048, 2048
N_HEAD = 16
HD = 64                       # per-component head dim (q1/k1/q2/k2)
DV = 128                      # v head dim
SCALE = HD ** -0.5
LAMBDA_INIT = 0.8 - 0.6 * math.exp(-0.3 * (1 - 1))
P = 128
NHC = 4                       # heads per core
CQ = NHC * DV                 # 512: per-core q'/k'/v width
N_CORES = 8
NE = E // P                   # 16 contraction chunks
NP8 = NE // 2                 # 8 e-pair chunks for DoubleRow
NS = T // P                   # 16 s chunks

# power-of-2 quantization scales (chosen for the spec's randn*0.02 weight /
# randn activation distributions with >=2x headroom in e4m3 and fp16)
SX = 16.0
SWQ = 512.0
SWK = 256.0
SWV = 2048.0
S_EXP = 1.0 / (SX * SX * SWQ * SWK)   # folds all q/k descales into exp()

TERMS = (("h", "h"), ("h", "l"), ("l", "h"))   # (x side, w side) fp8 terms

_NC_CACHE = None


def _build_nc():
    import concourse.mybir as mybir
    import concourse.tile as tile
    from concourse import bacc, bass_isa

    f32 = mybir.dt.float32
    bf16 = mybir.dt.bfloat16
    f16 = mybir.dt.float16
    fp8 = mybir.dt.float8e4
    DR = mybir.MatmulPerfMode.DoubleRow
    EXP = mybir.ActivationFunctionType.Exp

    nc = bacc.Bacc("TRN2", target_bir_lowering=False, debug=False,
                   num_devices=N_CORES)
    xd = {a: nc.dram_tensor(f"x{a}", [NP8, P, 2, T], fp8,
                            kind="ExternalInput").ap() for a in "hl"}
    wd = {(w, a): nc.dram_tensor(f"w{w}{a}", [P, NP8, 2, CQ], fp8,
                                 kind="ExternalInput").ap()
          for w in "qkv" for a in "hl"}
    wp16 = nc.dram_tensor("wp16", [P, NHC, E], bf16, kind="ExternalInput").ap()
    out = nc.dram_tensor("out", [T, E], f32, kind="ExternalOutput").ap()

    with tile.TileContext(nc) as tc:
        with (
            tc.tile_pool(name="res", bufs=1) as res,
            tc.tile_pool(name="pb", bufs=1) as pb,
            tc.tile_pool(name="pb_ps", bufs=1, space="PSUM") as pb_ps,
        ):
            qt = res.tile([P, NHC, T], f16, name="qt")     # q' * SX*SWQ
            kt = res.tile([P, NHC, T], f16, name="kt")     # k' * SX*SWK
            vsb = res.tile([P, NS, CQ], bf16, name="vsb")  # v * SX*SWV
            ot = res.tile([P, NHC, T], bf16, name="ot")    # O^T per head
            wpt = res.tile([P, NHC, E], bf16, name="wpt")
            nc.scalar.dma_start(wpt, wp16)

            # ---------- phase helpers ---------------------------------

            def emit_B(tb, h, s_hook=None):
                """Attention for (t-block tb, head h); s_hook(s) may inject
                extra PE work after each logits matmul pair."""
                t0 = tb * 1024
                pso = [
                    pb_ps.tile([P, 512], f32, name=f"pso{i}",
                               tag=f"pso{i}", bufs=1)
                    for i in range(2)
                ]
                acc2 = pb.tile([P, 1024], bf16, name="acc", tag="acc",
                               bufs=2)
                for s in range(NS):
                    psa2 = pb_ps.tile([P, 1024], f32, name="psa",
                                      tag="psa", bufs=2)
                    for half in range(2):
                        nc.tensor.matmul(
                            psa2[:, half * 512:(half + 1) * 512],
                            lhsT=kt[:, h, s * P:(s + 1) * P],
                            rhs=qt[:, h,
                                   t0 + half * 512:t0 + (half + 1) * 512],
                            start=True, stop=True,
                        )
                    if s_hook is not None:
                        s_hook(s)
                    et2 = pb.tile([P, 1024], bf16, name="et", tag="et",
                                  bufs=4)
                    nc.scalar.activation(et2, psa2, EXP, scale=S_EXP)
                    for half in range(2):
                        nc.tensor.matmul(
                            pso[half],
                            lhsT=vsb[:, s, h * P:(h + 1) * P],
                            rhs=et2[:, half * 512:(half + 1) * 512],
                            start=(s == 0), stop=(s == NS - 1),
                        )
                    if s == 0:
                        nc.vector.tensor_copy(acc2, et2)
                    else:
                        nc.vector.tensor_add(acc2, acc2, et2)
                zs = pb.tile([P, 1024], f32, name="zs", tag="zs", bufs=2)
                nc.gpsimd.partition_all_reduce(
                    zs, acc2, channels=P, reduce_op=bass_isa.ReduceOp.add)
                rb = pb.tile([P, 1024], f32, name="rb", tag="rb", bufs=2)
                nc.vector.reciprocal(rb, zs)
                for half in range(2):
                    nc.vector.tensor_mul(
                        ot[:, h, t0 + half * 512:t0 + (half + 1) * 512],
                        pso[half], rb[:, half * 512:(half + 1) * 512])

            def emit_D(tb, tj):
                tg = tb * 8 + tj
                for eo in range(4):
                    psd = pb_ps.tile([P, 512], f32, name="psd",
                                     tag="pqs", bufs=2)
                    for h in range(NHC):
                        nc.tensor.matmul(
                            psd,
                            lhsT=ot[:, h, tg * P:(tg + 1) * P],
                            rhs=wpt[:, h, eo * 512:(eo + 1) * 512],
                            start=(h == 0), stop=(h == NHC - 1),
                        )
                    osb = pb.tile([P, 512], f32, name="osb", tag="osb",
                                  bufs=3)
                    nc.vector.tensor_copy(osb, psd)
                    nc.sync.dma_start(
                        out[tg * P:(tg + 1) * P,
                            eo * 512:(eo + 1) * 512],
                        osb)

            # ---------- Phase A prefix + pipelined B/D -------------------
            with (
                tc.tile_pool(name="pa_w", bufs=1) as pa_w,
                tc.tile_pool(name="pa_x", bufs=1) as pa_x,
            ):
                wt = {}
                for w in "qkv":
                    for a in "hl":
                        wtile = pa_w.tile([P, NP8, 2, CQ], fp8,
                                          name=f"w{w}{a}", tag=f"w{w}{a}",
                                          bufs=1)
                        nc.sync.dma_start(wtile, wd[(w, a)])
                        wt[(w, a)] = wtile

                xe_blocks = [{}, {}]

                def get_xe(bo, pair, a):
                    # tags are shared between the two t-blocks (16 tags,
                    # bufs=1): block 1's DMA starts as soon as block 0's
                    # last reader of that tag is done
                    xe = xe_blocks[bo]
                    if (pair, a) not in xe:
                        tl = pa_x.tile([P, 2, 1024], fp8,
                                       name=f"xe{bo}{pair}{a}",
                                       tag=f"xe{pair}{a}", bufs=1)
                        nc.gpsimd.dma_start(
                            tl, xd[a][pair, :, :, bo * 1024:(bo + 1) * 1024])
                        xe[(pair, a)] = tl
                    return xe[(pair, a)]

                # A-round psum tiles borrow phase B's tag slots (2x[P,1024]
                # "psa" + pso0 + pso1 + 2x[P,512] "pqs" = exactly 8 banks);
                # the returned list holds 8 [P,512] views indexed c*2+half.
                def alloc_round_psums():
                    pa0 = pb_ps.tile([P, 1024], f32, name="pssA",
                                     tag="psa", bufs=2)
                    pa1 = pb_ps.tile([P, 1024], f32, name="pssB",
                                     tag="psa", bufs=2)
                    o0 = pb_ps.tile([P, 512], f32, name="pssC",
                                    tag="pso0", bufs=1)
                    o1 = pb_ps.tile([P, 512], f32, name="pssD",
                                    tag="pso1", bufs=1)
                    q0 = pb_ps.tile([P, 512], f32, name="pssE",
                                    tag="pqs", bufs=2)
                    q1 = pb_ps.tile([P, 512], f32, name="pssF",
                                    tag="pqs", bufs=2)
                    return [pa0[:, 0:512], pa0[:, 512:1024],
                            pa1[:, 0:512], pa1[:, 512:1024],
                            o0, o1, q0, q1]

                if True:
                    # --- A prefix: b0 [Q K V], b1 [K V] ---
                    def emit_qk_round(bo, w, dst):
                        t0 = bo * 1024
                        pss = alloc_round_psums()
                        for pair in range(NP8):
                            for ti, (xa, wb) in enumerate(TERMS):
                                xt = get_xe(bo, pair, xa)
                                wtile = wt[(w, wb)]
                                for c in range(4):
                                    for half in range(2):
                                        nc.tensor.matmul(
                                            pss[c * 2 + half],
                                            lhsT=wtile[:, pair, :,
                                                       c * P:(c + 1) * P],
                                            rhs=xt[:, :,
                                                   half * 512:(half + 1) * 512],
                                            perf_mode=DR,
                                            start=(pair == 0 and ti == 0),
                                            stop=(pair == NP8 - 1 and ti == 2),
                                        )
                        for c in range(4):
                            for half in range(2):
                                nc.vector.tensor_copy(
                                    dst[:, c,
                                        t0 + half * 512:t0 + (half + 1) * 512],
                                    pss[c * 2 + half])

                    def emit_v_round(bo):
                        psv = alloc_round_psums()
                        for pair in range(NP8):
                            for ti, (xa, wb) in enumerate(TERMS):
                                xt = get_xe(bo, pair, xa)
                                wtile = wt[("v", wb)]
                                for tj in range(8):
                                    nc.tensor.matmul(
                                        psv[tj],
                                        lhsT=xt[:, :, tj * P:(tj + 1) * P],
                                        rhs=wtile[:, pair, :, :],
                                        perf_mode=DR,
                                        start=(pair == 0 and ti == 0),
                                        stop=(pair == NP8 - 1 and ti == 2),
                                    )
                        # split the copies of the last A round between
                        # DVE and ACT so the trailing drain that gates
                        # phase B's first PSUM allocations halves
                        for tj in range(8):
                            dst = vsb[:, bo * 8 + tj, :]
                            if bo == 1 and tj % 2 == 0:
                                nc.scalar.copy(dst, psv[tj])
                            else:
                                nc.vector.tensor_copy(dst, psv[tj])

                    emit_qk_round(0, "q", qt)
                    emit_qk_round(0, "k", kt)
                    emit_v_round(0)
                    emit_qk_round(1, "k", kt)
                    emit_v_round(1)

                # --- B(tb0) with the b1-Q round injected 3 mm per s ---
                # sub-round h covers qt chunk c==h (heads line up with the
                # consumer B(tb1, h)); s-iters 0-7 accumulate half 0,
                # s-iters 8-15 half 1.
                for h in range(NHC):
                    state = {}

                    def q_hook(s, h=h, state=state):
                        half = s // 8
                        j = (s % 8) * 3           # 24 mms per half
                        if j == 0:
                            state[half] = pb_ps.tile(
                                [P, 512], f32, name="pqs", tag="pqs",
                                bufs=2)
                        pq = state[half]
                        for k in range(3):
                            idx = j + k
                            pair, ti = divmod(idx, 3)
                            xa, wb = TERMS[ti]
                            xt = get_xe(1, pair, xa)
                            nc.tensor.matmul(
                                pq,
                                lhsT=wt[("q", wb)][:, pair, :,
                                                   h * P:(h + 1) * P],
                                rhs=xt[:, :, half * 512:(half + 1) * 512],
                                perf_mode=DR,
                                start=(idx == 0), stop=(idx == 23),
                            )
                        if s % 8 == 7:
                            nc.vector.tensor_copy(
                                qt[:, h,
                                   1024 + half * 512:1024 + (half + 1) * 512],
                                pq)

                    emit_B(0, h, s_hook=q_hook)

            for h in range(NHC):
                emit_B(1, h)
                emit_D(0, 2 * h)
                emit_D(0, 2 * h + 1)
            for tj in range(8):
                emit_D(1, tj)

    nc.compile()
    return nc


def _get_nc():
    global _NC_CACHE
    if _NC_CACHE is None:
        _NC_CACHE = _build_nc()
    return _NC_CACHE


def _split8(a, s):
    """a*s ~= hi + lo, both e4m3 at a common scale; returns fp8 arrays."""
    import ml_dtypes
    f8 = ml_dtypes.float8_e4m3fn
    a = np.asarray(a, np.float32) * s
    hi = a.astype(f8)
    lo = (a - hi.astype(np.float32)).astype(f8)
    return hi, lo


def _interleave(a):
    """[E, M] -> [NP8, 128, 2, M] with e = pair*256 + i*128 + p."""
    M = a.shape[1]
    return np.ascontiguousarray(
        a.reshape(NP8, 2, P, M).transpose(0, 2, 1, 3))


def _interleave_w(a):
    """[E, M] -> [128, NP8, 2, M] with e = pair*256 + i*128 + p."""
    M = a.shape[1]
    return np.ascontiguousarray(
        a.reshape(NP8, 2, P, M).transpose(2, 0, 1, 3))


def _shard_inputs(x, W_attn, W_proj, lambda_q1, lambda_k1,
                  lambda_q2, lambda_k2):
    import ml_dtypes
    bf16 = ml_dtypes.bfloat16
    x = np.asarray(x, np.float32)
    W_attn = np.asarray(W_attn, np.float32)
    W_proj = np.asarray(W_proj, np.float32)
    lam = float(np.exp(np.dot(np.asarray(lambda_q1, np.float32),
                              np.asarray(lambda_k1, np.float32)))
                - np.exp(np.dot(np.asarray(lambda_q2, np.float32),
                                np.asarray(lambda_k2, np.float32)))
                + LAMBDA_INIT)
    Cb = E // 2  # 1024: q1/k1/q2/k2 block width in W_attn

    # x splits are shared by the 4 cores of each batch element
    xsplit = []
    for b in range(B):
        xh, xl = _split8(np.ascontiguousarray(x[b].T), SX)
        xsplit.append((_interleave(xh), _interleave(xl)))

    in_maps = []
    for c in range(N_CORES):
        b, hg = divmod(c, 4)
        heads = [4 * hg + j for j in range(NHC)]
        wq_c = np.empty((E, CQ), np.float32)
        wk_c = np.empty((E, CQ), np.float32)
        wv_c = np.empty((E, CQ), np.float32)
        wp_c = np.empty((CQ, E), np.float32)
        for j, h in enumerate(heads):
            wq_c[:, j * P:j * P + HD] = W_attn[:, h * HD:(h + 1) * HD] * SCALE
            wq_c[:, j * P + HD:(j + 1) * P] = (
                W_attn[:, 2 * Cb + h * HD:2 * Cb + (h + 1) * HD]
                * (-lam * SCALE))
            wk_c[:, j * P:j * P + HD] = W_attn[:, Cb + h * HD:Cb + (h + 1) * HD]
            wk_c[:, j * P + HD:(j + 1) * P] = (
                W_attn[:, 3 * Cb + h * HD:3 * Cb + (h + 1) * HD])
            wv_c[:, j * P:(j + 1) * P] = (
                W_attn[:, 4 * Cb + h * DV:4 * Cb + (h + 1) * DV])
            wp_c[j * P:(j + 1) * P, :] = (
                W_proj[h * DV:(h + 1) * DV, :]
                * ((1.0 - LAMBDA_INIT) / (SX * SWV)))
        wqh, wql = _split8(wq_c, SWQ)
        wkh, wkl = _split8(wk_c, SWK)
        wvh, wvl = _split8(wv_c, SWV)
        xh8, xl8 = xsplit[b]
        in_maps.append({
            "xh": xh8, "xl": xl8,
            "wqh": _interleave_w(wqh), "wql": _interleave_w(wql),
            "wkh": _interleave_w(wkh), "wkl": _interleave_w(wkl),
            "wvh": _interleave_w(wvh), "wvl": _interleave_w(wvl),
            "wp16": np.ascontiguousarray(
                wp_c.reshape(NHC, P, E).transpose(1, 0, 2)).astype(bf16),
        })
    return in_maps


def _run(inputs, trace=False):
    from concourse.bass_utils import run_bass_kernel_spmd
    nc = _get_nc()
    in_maps = _shard_inputs(**inputs)
    res = run_bass_kernel_spmd(nc, in_maps, list(range(N_CORES)),
                               trace=trace)
    out = np.zeros((B, T, E), np.float32)
    for c in range(N_CORES):
        out[c // 4] += res.results[c]["out"]
    return out, res


def kernel(x, W_attn, W_proj, lambda_q1, lambda_k1, lambda_q2, lambda_k2):
    out, _ = _run(dict(x=x, W_attn=W_attn, W_proj=W_proj,
                       lambda_q1=lambda_q1, lambda_k1=lambda_k1,
                       lambda_q2=lambda_q2, lambda_k2=lambda_k2))
    return out


# revision 13
# speedup vs baseline: 1.0388x; 1.0388x over previous
"""MultiHeadDiffAttention Trainium2 kernel, v3.

Strategy (8 NeuronCores, SPMD), same sharding as baseline:
  core c handles b = c//4, heads 4*(c%4)..4*(c%4)+3.

Perf structure (CoreSim-modeled, HW-verified numerics):
  1. QKV projection runs as error-compensated split-fp8 (e4m3 hi+lo with a
     shared power-of-2 scale, 3 of 4 cross terms) using DoubleRow matmuls:
     each instruction contracts 256 rows at 0.5 cycles/row, so the
     projection costs 0.75x the fp32r schedule at ~bf16 accuracy.
     x and W are split/interleaved host-side; all descale factors fold into
     the exp() scale and the (host-prepared) W_proj.
  2. The softmax denominator comes from a bf16 DVE accumulation of the exp
     tiles plus one Pool partition_all_reduce - no PE ones-matmuls.
  3. exp() runs 1024-wide (2 PSUM banks per logits tile) to halve ACT's
     per-instruction access overhead; q/k are stored fp16 (smaller scales)
     so the large-logit heads keep ~1e-3 score accuracy.
  4. Software pipelining: the t-block-1 Q projection is injected 3 matmuls
     per s-iteration into the first attention block (which is otherwise
     ACT-bound), and the output projection of block 0 is interleaved with
     the second attention block, so PE never drains between phases.
"""

import math

import numpy as np

B, T, E = 2, 2048, 2048
N_HEAD = 16
HD = 64                       # per-component head dim (q1/k1/q2/k2)
DV = 128                      # v head dim
SCALE = HD ** -0.5
LAMBDA_INIT = 0.8 - 0.6 * math.exp(-0.3 * (1 - 1))
P = 128
NHC = 4                       # heads per core
CQ = NHC * DV                 # 512: per-core q'/k'/v width
N_CORES = 8
NE = E // P                   # 16 contraction chunks
NP8 = NE // 2                 # 8 e-pair chunks for DoubleRow
NS = T // P                   # 16 s chunks

# power-of-2 quantization scales (chosen for the spec's randn*0.02 weight /
# randn activation distributions with >=2x headroom in e4m3 and fp16)
SX = 16.0
SWQ = 512.0
SWK = 256.0
SWV = 1024.0
S_EXP = 1.0 / (SX * SX * SWQ * SWK)   # folds all q/k descales into exp()
S_OT = 1.0 / 512.0                     # ot -> fp8 hi/lo scale
SWP = float(2 ** 22)                  # W_proj fp8 scale (after v-descale fold)
S_OUT = 1.0 / (S_OT * SWP)            # final output descale

TERMS = (("h", "h"), ("h", "l"), ("l", "h"))   # (x side, w side) fp8 terms

_NC_CACHE = None


def _build_nc():
    import concourse.mybir as mybir
    import concourse.tile as tile
    from concourse import bacc, bass_isa

    f32 = mybir.dt.float32
    bf16 = mybir.dt.bfloat16
    f16 = mybir.dt.float16
    fp8 = mybir.dt.float8e4
    DR = mybir.MatmulPerfMode.DoubleRow
    EXP = mybir.ActivationFunctionType.Exp
    COPY = mybir.ActivationFunctionType.Copy

    nc = bacc.Bacc("TRN2", target_bir_lowering=False, debug=False,
                   num_devices=N_CORES)
    xd = {a: nc.dram_tensor(f"x{a}", [NP8, P, 2, T], fp8,
                            kind="ExternalInput").ap() for a in "hl"}
    wd = {(w, a): nc.dram_tensor(f"w{w}{a}", [P, NP8, 2, CQ], fp8,
                                 kind="ExternalInput").ap()
          for w in "qkv" for a in "hl"}
    wpd = {a: nc.dram_tensor(f"wp{a}", [P, 2, 2, E], fp8,
                             kind="ExternalInput").ap() for a in "hl"}
    out = nc.dram_tensor("out", [T, E], bf16, kind="ExternalOutput").ap()

    with tile.TileContext(nc) as tc:
        with (
            tc.tile_pool(name="res", bufs=1) as res,
            tc.tile_pool(name="pb", bufs=1) as pb,
            tc.tile_pool(name="pb_ps", bufs=1, space="PSUM") as pb_ps,
        ):
            qt = res.tile([P, NHC, T], f16, name="qt")     # q' * SX*SWQ
            kt = res.tile([P, NHC, T], f16, name="kt")     # k' * SX*SWK
            vsb = res.tile([P, NS, CQ], bf16, name="vsb")  # v * SX*SWV
            ot = res.tile([P, NHC, T], bf16, name="ot")    # O^T per head
            oth = res.tile([P, NHC, T], fp8, name="oth")   # ot*S_OT hi
            otl = res.tile([P, NHC, T], fp8, name="otl")   # ot*S_OT lo
            wpt8 = {}
            for a in "hl":
                wpt8[a] = res.tile([P, 2, 2, E], fp8, name=f"wpt{a}")
                nc.scalar.dma_start(wpt8[a], wpd[a])

            # ---------- phase helpers ---------------------------------

            def emit_B(tb, h, s_hook=None):
                """Attention for (t-block tb, head h); s_hook(s) may inject
                extra PE work after each logits matmul pair."""
                t0 = tb * 1024
                pso = [
                    pb_ps.tile([P, 512], f32, name=f"pso{i}",
                               tag=f"pso{i}", bufs=1)
                    for i in range(2)
                ]
                acc2 = pb.tile([P, 1024], bf16, name="acc", tag="acc",
                               bufs=2)
                for s in range(NS):
                    psa2 = pb_ps.tile([P, 1024], f32, name="psa",
                                      tag="psa", bufs=2)
                    for half in range(2):
                        nc.tensor.matmul(
                            psa2[:, half * 512:(half + 1) * 512],
                            lhsT=kt[:, h, s * P:(s + 1) * P],
                            rhs=qt[:, h,
                                   t0 + half * 512:t0 + (half + 1) * 512],
                            start=True, stop=True,
                        )
                    if s_hook is not None:
                        s_hook(s)
                    et2 = pb.tile([P, 1024], bf16, name="et", tag="et",
                                  bufs=4)
                    nc.scalar.activation(et2, psa2, EXP, scale=S_EXP)
                    for half in range(2):
                        nc.tensor.matmul(
                            pso[half],
                            lhsT=vsb[:, s, h * P:(h + 1) * P],
                            rhs=et2[:, half * 512:(half + 1) * 512],
                            start=(s == 0), stop=(s == NS - 1),
                        )
                    if s == 0:
                        nc.vector.tensor_copy(acc2, et2)
                    else:
                        nc.vector.tensor_add(acc2, acc2, et2)
                zs = pb.tile([P, 1024], f32, name="zs", tag="zs", bufs=1)
                nc.gpsimd.partition_all_reduce(
                    zs, acc2, channels=P, reduce_op=bass_isa.ReduceOp.add)
                rb = pb.tile([P, 1024], f32, name="rb", tag="rb", bufs=2)
                nc.vector.reciprocal(rb, zs)
                for half in range(2):
                    nc.vector.tensor_mul(
                        ot[:, h, t0 + half * 512:t0 + (half + 1) * 512],
                        pso[half], rb[:, half * 512:(half + 1) * 512])
                osl = (slice(None), h, slice(t0, t0 + 1024))
                nc.scalar.activation(oth[osl], ot[osl], COPY, scale=S_OT)
                nc.vector.scalar_tensor_tensor(
                    otl[osl], ot[osl], S_OT, oth[osl],
                    mybir.AluOpType.mult, mybir.AluOpType.subtract)

            D_TERMS = ((oth, "h"), (oth, "l"), (otl, "h"))

            def d_mms(psd, tg, eo, jp):
                for ti, (osrc, wb) in enumerate(D_TERMS):
                    nc.tensor.matmul(
                        psd,
                        lhsT=osrc[:, 2 * jp:2 * jp + 2,
                                  tg * P:(tg + 1) * P],
                        rhs=wpt8[wb][:, jp, :, eo * 512:(eo + 1) * 512],
                        perf_mode=DR,
                        start=(jp == 0 and ti == 0),
                        stop=(jp == 1 and ti == 2),
                    )

            def emit_D(tb, tj, pd_pool):
                tg = tb * 8 + tj
                osb4 = pd_pool.tile([P, 4, 512], bf16, name="osb4",
                                    tag="osb4", bufs=4)
                for eo in range(4):
                    psd = pb_ps.tile([P, 512], f32, name="psd",
                                     tag="pqs", bufs=2)
                    d_mms(psd, tg, eo, 0)
                    d_mms(psd, tg, eo, 1)
                    # descale happens host-side; split staging copies so
                    # neither ACT (exp stream) nor DVE saturates
                    if eo == 1:
                        nc.scalar.copy(osb4[:, eo, :], psd)
                    else:
                        nc.vector.tensor_copy(osb4[:, eo, :], psd)
                dma_eng = nc.sync if tj % 2 == 0 else nc.gpsimd
                dma_eng.dma_start(out[tg * P:(tg + 1) * P, :], osb4)

            def emit_D1_all(pd_pool):
                # tail block: rotate psum through every free tag (the
                # attention tags are done) and run jp0 of the next groups
                # ahead of jp1 (which waits on the last head's normalize)
                slots = (("pqs", 2), ("pqs", 2), ("pso0", 1), ("pso1", 1),
                         ("psa", 2), ("psa", 2))
                groups = [(8 + tj, eo) for tj in range(8)
                          for eo in range(4)]
                DEPTH = 4
                osb = {}
                pend = []

                def finish(psd, tg, eo, gi):
                    d_mms(psd, tg, eo, 1)
                    tj = tg - 8
                    if tj not in osb:
                        osb[tj] = pd_pool.tile([P, 4, 512], bf16,
                                               name="osb4", tag="osb4",
                                               bufs=4)
                    if gi % 2:
                        nc.scalar.copy(osb[tj][:, eo, :], psd)
                    else:
                        nc.vector.tensor_copy(osb[tj][:, eo, :], psd)
                    if eo == 3:
                        eng = (nc.sync, nc.scalar, nc.gpsimd)[tj % 3]
                        eng.dma_start(out[tg * P:(tg + 1) * P, :],
                                      osb.pop(tj))

                for gi, (tg, eo) in enumerate(groups):
                    tag, bufs = slots[gi % len(slots)]
                    psd = pb_ps.tile([P, 512], f32, name="psd1",
                                     tag=tag, bufs=bufs)
                    d_mms(psd, tg, eo, 0)
                    pend.append((psd, tg, eo, gi))
                    if len(pend) >= DEPTH:
                        finish(*pend.pop(0))
                for args in pend:
                    finish(*args)

            # ---------- Phase A prefix + pipelined B/D -------------------
            with (
                tc.tile_pool(name="pa_w", bufs=1) as pa_w,
                tc.tile_pool(name="pa_x", bufs=1) as pa_x,
            ):
                wt = {}
                for w in "qkv":
                    for a in "hl":
                        wt[(w, a)] = pa_w.tile([P, NP8, 2, CQ], fp8,
                                               name=f"w{w}{a}",
                                               tag=f"w{w}{a}", bufs=1)
                for hp in range(2):
                    for w in "qkv":
                        for a in "hl":
                            nc.sync.dma_start(
                                wt[(w, a)][:, hp * 4:(hp + 1) * 4],
                                wd[(w, a)][:, hp * 4:(hp + 1) * 4])

                xe_blocks = [{}, {}]

                def get_xe(bo, pair, a):
                    # tags are shared between the two t-blocks (16 tags,
                    # bufs=1): block 1's DMA starts as soon as block 0's
                    # last reader of that tag is done
                    xe = xe_blocks[bo]
                    if (pair, a) not in xe:
                        tl = pa_x.tile([P, 2, 1024], fp8,
                                       name=f"xe{bo}{pair}{a}",
                                       tag=f"xe{pair}{a}", bufs=1)
                        nc.gpsimd.dma_start(
                            tl, xd[a][pair, :, :, bo * 1024:(bo + 1) * 1024])
                        xe[(pair, a)] = tl
                    return xe[(pair, a)]

                # A-round psum tiles borrow phase B's tag slots (2x[P,1024]
                # "psa" + pso0 + pso1 + 2x[P,512] "pqs" = exactly 8 banks);
                # the returned list holds 8 [P,512] views indexed c*2+half.
                def alloc_round_psums():
                    pa0 = pb_ps.tile([P, 1024], f32, name="pssA",
                                     tag="psa", bufs=2)
                    pa1 = pb_ps.tile([P, 1024], f32, name="pssB",
                                     tag="psa", bufs=2)
                    o0 = pb_ps.tile([P, 512], f32, name="pssC",
                                    tag="pso0", bufs=1)
                    o1 = pb_ps.tile([P, 512], f32, name="pssD",
                                    tag="pso1", bufs=1)
                    q0 = pb_ps.tile([P, 512], f32, name="pssE",
                                    tag="pqs", bufs=2)
                    q1 = pb_ps.tile([P, 512], f32, name="pssF",
                                    tag="pqs", bufs=2)
                    return [pa0[:, 0:512], pa0[:, 512:1024],
                            pa1[:, 0:512], pa1[:, 512:1024],
                            o0, o1, q0, q1]

                if True:
                    # --- A prefix: b0 [Q K V], b1 [K V] ---
                    def emit_qk_round(bo, w, dst):
                        t0 = bo * 1024
                        pss = alloc_round_psums()
                        for pair in range(NP8):
                            for ti, (xa, wb) in enumerate(TERMS):
                                xt = get_xe(bo, pair, xa)
                                wtile = wt[(w, wb)]
                                for c in range(4):
                                    for half in range(2):
                                        nc.tensor.matmul(
                                            pss[c * 2 + half],
                                            lhsT=wtile[:, pair, :,
                                                       c * P:(c + 1) * P],
                                            rhs=xt[:, :,
                                                   half * 512:(half + 1) * 512],
                                            perf_mode=DR,
                                            start=(pair == 0 and ti == 0),
                                            stop=(pair == NP8 - 1 and ti == 2),
                                        )
                        for c in range(4):
                            for half in range(2):
                                d_ = dst[:, c,
                                         t0 + half * 512:t0 + (half + 1) * 512]
                                if (c * 2 + half) % 2 == 0:
                                    nc.scalar.copy(d_, pss[c * 2 + half])
                                else:
                                    nc.vector.tensor_copy(d_, pss[c * 2 + half])

                    def emit_v_round(bo):
                        psv = alloc_round_psums()
                        for pair in range(NP8):
                            for ti, (xa, wb) in enumerate(TERMS):
                                xt = get_xe(bo, pair, xa)
                                wtile = wt[("v", wb)]
                                for tj in range(8):
                                    nc.tensor.matmul(
                                        psv[tj],
                                        lhsT=xt[:, :, tj * P:(tj + 1) * P],
                                        rhs=wtile[:, pair, :, :],
                                        perf_mode=DR,
                                        start=(pair == 0 and ti == 0),
                                        stop=(pair == NP8 - 1 and ti == 2),
                                    )
                        # split the copies of the last A round between
                        # DVE and ACT so the trailing drain that gates
                        # phase B's first PSUM allocations halves
                        for tj in range(8):
                            dst = vsb[:, bo * 8 + tj, :]
                            if tj % 2 == 0:
                                nc.scalar.copy(dst, psv[tj])
                            else:
                                nc.vector.tensor_copy(dst, psv[tj])

                    emit_qk_round(0, "q", qt)
                    emit_qk_round(0, "k", kt)
                    emit_v_round(0)
                    emit_qk_round(1, "k", kt)
                    emit_v_round(1)

                # --- B(tb0) with the b1-Q round injected 3 mm per s ---
                # sub-round h covers qt chunk c==h (heads line up with the
                # consumer B(tb1, h)); s-iters 0-7 accumulate half 0,
                # s-iters 8-15 half 1.
                for h in range(NHC):
                    state = {}

                    def q_hook(s, h=h, state=state):
                        half = s // 8
                        j = (s % 8) * 3           # 24 mms per half
                        if j == 0:
                            state[half] = pb_ps.tile(
                                [P, 512], f32, name="pqs", tag="pqs",
                                bufs=2)
                        pq = state[half]
                        for k in range(3):
                            idx = j + k
                            pair, ti = divmod(idx, 3)
                            xa, wb = TERMS[ti]
                            xt = get_xe(1, pair, xa)
                            nc.tensor.matmul(
                                pq,
                                lhsT=wt[("q", wb)][:, pair, :,
                                                   h * P:(h + 1) * P],
                                rhs=xt[:, :, half * 512:(half + 1) * 512],
                                perf_mode=DR,
                                start=(idx == 0), stop=(idx == 23),
                            )
                        if s % 8 == 7:
                            nc.vector.tensor_copy(
                                qt[:, h,
                                   1024 + half * 512:1024 + (half + 1) * 512],
                                pq)

                    emit_B(0, h, s_hook=q_hook)

            with tc.tile_pool(name="pd", bufs=1) as pd_pool:
                for h in range(NHC):
                    emit_B(1, h)
                    emit_D(0, 2 * h, pd_pool)
                    emit_D(0, 2 * h + 1, pd_pool)
                emit_D1_all(pd_pool)

    nc.compile()
    return nc


def _get_nc():
    global _NC_CACHE
    if _NC_CACHE is None:
        _NC_CACHE = _build_nc()
    return _NC_CACHE


def _split8(a, s):
    """a*s ~= hi + lo, both e4m3 at a common scale; returns fp8 arrays."""
    import ml_dtypes
    f8 = ml_dtypes.float8_e4m3fn
    a = np.asarray(a, np.float32) * s
    amax = float(np.abs(a).max())
    assert amax < 200.0, f"fp8 range: scaled absmax {amax}"
    hi = a.astype(f8)
    lo = (a - hi.astype(np.float32)).astype(f8)
    return hi, lo


def _interleave(a):
    """[E, M] -> [NP8, 128, 2, M] with e = pair*256 + i*128 + p."""
    M = a.shape[1]
    return np.ascontiguousarray(
        a.reshape(NP8, 2, P, M).transpose(0, 2, 1, 3))


def _interleave_w(a):
    """[E, M] -> [128, NP8, 2, M] with e = pair*256 + i*128 + p."""
    M = a.shape[1]
    return np.ascontiguousarray(
        a.reshape(NP8, 2, P, M).transpose(2, 0, 1, 3))


def _shard_inputs(x, W_attn, W_proj, lambda_q1, lambda_k1,
                  lambda_q2, lambda_k2):
    import ml_dtypes
    bf16 = ml_dtypes.bfloat16
    x = np.asarray(x, np.float32)
    W_attn = np.asarray(W_attn, np.float32)
    W_proj = np.asarray(W_proj, np.float32)
    lam = float(np.exp(np.dot(np.asarray(lambda_q1, np.float32),
                              np.asarray(lambda_k1, np.float32)))
                - np.exp(np.dot(np.asarray(lambda_q2, np.float32),
                                np.asarray(lambda_k2, np.float32)))
                + LAMBDA_INIT)
    Cb = E // 2  # 1024: q1/k1/q2/k2 block width in W_attn

    # x splits are shared by the 4 cores of each batch element
    xsplit = []
    for b in range(B):
        xh, xl = _split8(np.ascontiguousarray(x[b].T), SX)
        xsplit.append((_interleave(xh), _interleave(xl)))

    in_maps = []
    for c in range(N_CORES):
        b, hg = divmod(c, 4)
        heads = [4 * hg + j for j in range(NHC)]
        wq_c = np.empty((E, CQ), np.float32)
        wk_c = np.empty((E, CQ), np.float32)
        wv_c = np.empty((E, CQ), np.float32)
        wp_c = np.empty((CQ, E), np.float32)
        for j, h in enumerate(heads):
            wq_c[:, j * P:j * P + HD] = W_attn[:, h * HD:(h + 1) * HD] * SCALE
            wq_c[:, j * P + HD:(j + 1) * P] = (
                W_attn[:, 2 * Cb + h * HD:2 * Cb + (h + 1) * HD]
                * (-lam * SCALE))
            wk_c[:, j * P:j * P + HD] = W_attn[:, Cb + h * HD:Cb + (h + 1) * HD]
            wk_c[:, j * P + HD:(j + 1) * P] = (
                W_attn[:, 3 * Cb + h * HD:3 * Cb + (h + 1) * HD])
            wv_c[:, j * P:(j + 1) * P] = (
                W_attn[:, 4 * Cb + h * DV:4 * Cb + (h + 1) * DV])
            wp_c[j * P:(j + 1) * P, :] = (
                W_proj[h * DV:(h + 1) * DV, :]
                * ((1.0 - LAMBDA_INIT) / (SX * SWV)))
        wqh, wql = _split8(wq_c, SWQ)
        wkh, wkl = _split8(wk_c, SWK)
        wvh, wvl = _split8(wv_c, SWV)
        wph, wpl = _split8(wp_c, SWP)

        def _ilv_p(a):
            # [CQ, E] -> [P, 2jp, 2i, E] with dv = jp*256 + i*128 + p
            return np.ascontiguousarray(
                a.reshape(2, 2, P, E).transpose(2, 0, 1, 3))

        xh8, xl8 = xsplit[b]
        in_maps.append({
            "xh": xh8, "xl": xl8,
            "wqh": _interleave_w(wqh), "wql": _interleave_w(wql),
            "wkh": _interleave_w(wkh), "wkl": _interleave_w(wkl),
            "wvh": _interleave_w(wvh), "wvl": _interleave_w(wvl),
            "wph": _ilv_p(wph), "wpl": _ilv_p(wpl),
        })
    return in_maps


def _run(inputs, trace=False):
    from concourse.bass_utils import run_bass_kernel_spmd
    nc = _get_nc()
    in_maps = _shard_inputs(**inputs)
    res = run_bass_kernel_spmd(nc, in_maps, list(range(N_CORES)),
                               trace=trace)
    out = np.zeros((B, T, E), np.float32)
    for c in range(N_CORES):
        out[c // 4] += res.results[c]["out"].astype(np.float32)
    out *= S_OUT
    return out, res


def kernel(x, W_attn, W_proj, lambda_q1, lambda_k1, lambda_q2, lambda_k2):
    out, _ = _run(dict(x=x, W_attn=W_attn, W_proj=W_proj,
                       lambda_q1=lambda_q1, lambda_k1=lambda_k1,
                       lambda_q2=lambda_q2, lambda_k2=lambda_k2))
    return out


# revision 14
# speedup vs baseline: 1.0513x; 1.0120x over previous
"""MultiHeadDiffAttention Trainium2 kernel, v3.

Strategy (8 NeuronCores, SPMD), same sharding as baseline:
  core c handles b = c//4, heads 4*(c%4)..4*(c%4)+3.

Perf structure (CoreSim-modeled, HW-verified numerics):
  1. QKV projection runs as error-compensated split-fp8 (e4m3 hi+lo with a
     shared power-of-2 scale, 3 of 4 cross terms) using DoubleRow matmuls:
     each instruction contracts 256 rows at 0.5 cycles/row, so the
     projection costs 0.75x the fp32r schedule at ~bf16 accuracy.
     x and W are split/interleaved host-side; all descale factors fold into
     the exp() scale and the (host-prepared) W_proj.
  2. The softmax denominator comes from a bf16 DVE accumulation of the exp
     tiles plus one Pool partition_all_reduce - no PE ones-matmuls.
  3. exp() runs 1024-wide (2 PSUM banks per logits tile) to halve ACT's
     per-instruction access overhead; q/k are stored fp16 (smaller scales)
     so the large-logit heads keep ~1e-3 score accuracy.
  4. Software pipelining: the t-block-1 Q projection is injected 3 matmuls
     per s-iteration into the first attention block (which is otherwise
     ACT-bound), and the output projection of block 0 is interleaved with
     the second attention block, so PE never drains between phases.
"""

import math

import numpy as np

B, T, E = 2, 2048, 2048
N_HEAD = 16
HD = 64                       # per-component head dim (q1/k1/q2/k2)
DV = 128                      # v head dim
SCALE = HD ** -0.5
LAMBDA_INIT = 0.8 - 0.6 * math.exp(-0.3 * (1 - 1))
P = 128
NHC = 4                       # heads per core
CQ = NHC * DV                 # 512: per-core q'/k'/v width
N_CORES = 8
NE = E // P                   # 16 contraction chunks
NP8 = NE // 2                 # 8 e-pair chunks for DoubleRow
NS = T // P                   # 16 s chunks

# power-of-2 quantization scales (chosen for the spec's randn*0.02 weight /
# randn activation distributions with >=2x headroom in e4m3 and fp16)
SX = 16.0
SWQ = 512.0
SWK = 256.0
SWV = 1024.0
S_EXP = 1.0 / (SX * SX * SWQ * SWK)   # folds all q/k descales into exp()
S_OT = 1.0 / 512.0                     # ot -> fp8 hi/lo scale
SWP = float(2 ** 22)                  # W_proj fp8 scale (after v-descale fold)
S_OUT = 1.0 / (S_OT * SWP)            # final output descale

TERMS = (("h", "h"), ("h", "l"), ("l", "h"))   # (x side, w side) fp8 terms

_NC_CACHE = None


def _build_nc():
    import concourse.mybir as mybir
    import concourse.tile as tile
    from concourse import bacc, bass_isa

    f32 = mybir.dt.float32
    bf16 = mybir.dt.bfloat16
    f16 = mybir.dt.float16
    fp8 = mybir.dt.float8e4
    DR = mybir.MatmulPerfMode.DoubleRow
    EXP = mybir.ActivationFunctionType.Exp
    COPY = mybir.ActivationFunctionType.Copy

    nc = bacc.Bacc("TRN2", target_bir_lowering=False, debug=False,
                   num_devices=N_CORES)
    xd = {a: nc.dram_tensor(f"x{a}", [NP8, P, 2, T], fp8,
                            kind="ExternalInput").ap() for a in "hl"}
    wd = {(w, a): nc.dram_tensor(f"w{w}{a}", [P, NP8, 2, CQ], fp8,
                                 kind="ExternalInput").ap()
          for w in "qkv" for a in "hl"}
    wpd = {a: nc.dram_tensor(f"wp{a}", [P, 2, 2, E], fp8,
                             kind="ExternalInput").ap() for a in "hl"}
    out = nc.dram_tensor("out", [T, E], bf16, kind="ExternalOutput").ap()

    with tile.TileContext(nc) as tc:
        with (
            tc.tile_pool(name="res", bufs=1) as res,
            tc.tile_pool(name="pb", bufs=1) as pb,
            tc.tile_pool(name="pb_ps", bufs=1, space="PSUM") as pb_ps,
        ):
            qt = res.tile([P, NHC, T], f16, name="qt")     # q' * SX*SWQ
            kt = res.tile([P, NHC, T], f16, name="kt")     # k' * SX*SWK
            vsb = res.tile([P, NS, CQ], bf16, name="vsb")  # v * SX*SWV
            ot = res.tile([P, NHC, T], bf16, name="ot")    # O^T per head
            oth = res.tile([P, NHC, T], fp8, name="oth")   # ot*S_OT hi
            otl = res.tile([P, NHC, T], fp8, name="otl")   # ot*S_OT lo
            wpt8 = {}
            for a in "hl":
                wpt8[a] = res.tile([P, 2, 2, E], fp8, name=f"wpt{a}")
                nc.scalar.dma_start(wpt8[a], wpd[a])

            # ---------- phase helpers ---------------------------------

            def emit_B(tb, h, s_hook=None):
                """Attention for (t-block tb, head h); s_hook(s) may inject
                extra PE work after each logits matmul pair."""
                t0 = tb * 1024
                pso = [
                    pb_ps.tile([P, 512], f32, name=f"pso{i}",
                               tag=f"pso{i}", bufs=1)
                    for i in range(2)
                ]
                acc2 = pb.tile([P, 1024], bf16, name="acc", tag="acc",
                               bufs=2)
                for s in range(NS):
                    psa2 = pb_ps.tile([P, 1024], f32, name="psa",
                                      tag="psa", bufs=2)
                    for half in range(2):
                        nc.tensor.matmul(
                            psa2[:, half * 512:(half + 1) * 512],
                            lhsT=kt[:, h, s * P:(s + 1) * P],
                            rhs=qt[:, h,
                                   t0 + half * 512:t0 + (half + 1) * 512],
                            start=True, stop=True,
                        )
                    if s_hook is not None:
                        s_hook(s)
                    et2 = pb.tile([P, 1024], bf16, name="et", tag="et",
                                  bufs=4)
                    nc.scalar.activation(et2, psa2, EXP, scale=S_EXP)
                    for half in range(2):
                        nc.tensor.matmul(
                            pso[half],
                            lhsT=vsb[:, s, h * P:(h + 1) * P],
                            rhs=et2[:, half * 512:(half + 1) * 512],
                            start=(s == 0), stop=(s == NS - 1),
                        )
                    if s == 0:
                        nc.vector.tensor_copy(acc2, et2)
                    else:
                        nc.vector.tensor_add(acc2, acc2, et2)
                zs = pb.tile([P, 1024], f32, name="zs", tag="zs", bufs=1)
                nc.gpsimd.partition_all_reduce(
                    zs, acc2, channels=P, reduce_op=bass_isa.ReduceOp.add)
                rb = pb.tile([P, 1024], f32, name="rb", tag="rb", bufs=2)
                nc.vector.reciprocal(rb, zs)
                for half in range(2):
                    nc.vector.tensor_mul(
                        ot[:, h, t0 + half * 512:t0 + (half + 1) * 512],
                        pso[half], rb[:, half * 512:(half + 1) * 512])
                osl = (slice(None), h, slice(t0, t0 + 1024))
                nc.scalar.activation(oth[osl], ot[osl], COPY, scale=S_OT)
                nc.vector.scalar_tensor_tensor(
                    otl[osl], ot[osl], S_OT, oth[osl],
                    mybir.AluOpType.mult, mybir.AluOpType.subtract)

            D_TERMS = ((oth, "h"), (oth, "l"), (otl, "h"))

            def d_mms(psd, tg, eo, jp):
                for ti, (osrc, wb) in enumerate(D_TERMS):
                    nc.tensor.matmul(
                        psd,
                        lhsT=osrc[:, 2 * jp:2 * jp + 2,
                                  tg * P:(tg + 1) * P],
                        rhs=wpt8[wb][:, jp, :, eo * 512:(eo + 1) * 512],
                        perf_mode=DR,
                        start=(jp == 0 and ti == 0),
                        stop=(jp == 1 and ti == 2),
                    )

            def emit_D(tb, tj, pd_pool):
                tg = tb * 8 + tj
                osb4 = pd_pool.tile([P, 4, 512], bf16, name="osb4",
                                    tag="osb4", bufs=4)
                for eo in range(4):
                    psd = pb_ps.tile([P, 512], f32, name="psd",
                                     tag="pqs", bufs=2)
                    d_mms(psd, tg, eo, 0)
                    d_mms(psd, tg, eo, 1)
                    # descale happens host-side; split staging copies so
                    # neither ACT (exp stream) nor DVE saturates
                    if eo == 1:
                        nc.scalar.copy(osb4[:, eo, :], psd)
                    else:
                        nc.vector.tensor_copy(osb4[:, eo, :], psd)
                dma_eng = nc.sync if tj % 2 == 0 else nc.gpsimd
                dma_eng.dma_start(out[tg * P:(tg + 1) * P, :], osb4)

            def emit_D1_all(pd_pool):
                # tail block: rotate psum through every free tag (the
                # attention tags are done) and run jp0 of the next groups
                # ahead of jp1 (which waits on the last head's normalize)
                slots = (("pqs", 2), ("pqs", 2), ("pso0", 1), ("pso1", 1),
                         ("psa", 2), ("psa", 2))
                groups = [(8 + tj, eo) for tj in range(8)
                          for eo in range(4)]
                DEPTH = 5
                osb = {}
                pend = []

                def finish(psd, tg, eo, gi):
                    d_mms(psd, tg, eo, 1)
                    tj = tg - 8
                    if tj not in osb:
                        osb[tj] = pd_pool.tile([P, 4, 512], bf16,
                                               name="osb4", tag="osb4",
                                               bufs=4)
                    if gi % 2:
                        nc.scalar.copy(osb[tj][:, eo, :], psd)
                    else:
                        nc.vector.tensor_copy(osb[tj][:, eo, :], psd)
                    if eo == 3:
                        ob = osb.pop(tj)
                        if tj >= 6:
                            for k, eng in enumerate(
                                    (nc.sync, nc.scalar, nc.gpsimd,
                                     nc.sync)):
                                eng.dma_start(
                                    out[tg * P:(tg + 1) * P,
                                        k * 512:(k + 1) * 512],
                                    ob[:, k, :])
                        else:
                            eng = (nc.sync, nc.scalar, nc.gpsimd)[tj % 3]
                            eng.dma_start(out[tg * P:(tg + 1) * P, :], ob)

                for gi, (tg, eo) in enumerate(groups):
                    tag, bufs = slots[gi % len(slots)]
                    psd = pb_ps.tile([P, 512], f32, name="psd1",
                                     tag=tag, bufs=bufs)
                    d_mms(psd, tg, eo, 0)
                    pend.append((psd, tg, eo, gi))
                    if len(pend) >= DEPTH:
                        finish(*pend.pop(0))
                for args in pend:
                    finish(*args)

            # ---------- Phase A prefix + pipelined B/D -------------------
            with (
                tc.tile_pool(name="pa_w", bufs=1) as pa_w,
                tc.tile_pool(name="pa_x", bufs=1) as pa_x,
            ):
                wt = {}
                for w in "qkv":
                    for a in "hl":
                        wt[(w, a)] = pa_w.tile([P, NP8, 2, CQ], fp8,
                                               name=f"w{w}{a}",
                                               tag=f"w{w}{a}", bufs=1)
                for a in "hl":
                    nc.sync.dma_start(wt[("q", a)][:, 0:1],
                                      wd[("q", a)][:, 0:1])
                for hp, lo, hi in ((0, 1, 4), (1, 4, 8)):
                    for w in "qkv":
                        for a in "hl":
                            if w == "q":
                                nc.sync.dma_start(wt[(w, a)][:, lo:hi],
                                                  wd[(w, a)][:, lo:hi])
                            else:
                                nc.sync.dma_start(
                                    wt[(w, a)][:, hp * 4:(hp + 1) * 4],
                                    wd[(w, a)][:, hp * 4:(hp + 1) * 4])

                xe_blocks = [{}, {}]

                def get_xe(bo, pair, a):
                    # tags are shared between the two t-blocks (16 tags,
                    # bufs=1): block 1's DMA starts as soon as block 0's
                    # last reader of that tag is done
                    xe = xe_blocks[bo]
                    if (pair, a) not in xe:
                        tl = pa_x.tile([P, 2, 1024], fp8,
                                       name=f"xe{bo}{pair}{a}",
                                       tag=f"xe{pair}{a}", bufs=1)
                        nc.gpsimd.dma_start(
                            tl, xd[a][pair, :, :, bo * 1024:(bo + 1) * 1024])
                        xe[(pair, a)] = tl
                    return xe[(pair, a)]

                # A-round psum tiles borrow phase B's tag slots (2x[P,1024]
                # "psa" + pso0 + pso1 + 2x[P,512] "pqs" = exactly 8 banks);
                # the returned list holds 8 [P,512] views indexed c*2+half.
                def alloc_round_psums():
                    pa0 = pb_ps.tile([P, 1024], f32, name="pssA",
                                     tag="psa", bufs=2)
                    pa1 = pb_ps.tile([P, 1024], f32, name="pssB",
                                     tag="psa", bufs=2)
                    o0 = pb_ps.tile([P, 512], f32, name="pssC",
                                    tag="pso0", bufs=1)
                    o1 = pb_ps.tile([P, 512], f32, name="pssD",
                                    tag="pso1", bufs=1)
                    q0 = pb_ps.tile([P, 512], f32, name="pssE",
                                    tag="pqs", bufs=2)
                    q1 = pb_ps.tile([P, 512], f32, name="pssF",
                                    tag="pqs", bufs=2)
                    return [pa0[:, 0:512], pa0[:, 512:1024],
                            pa1[:, 0:512], pa1[:, 512:1024],
                            o0, o1, q0, q1]

                if True:
                    # --- A prefix: b0 [Q K V], b1 [K V] ---
                    def emit_qk_round(bo, w, dst):
                        t0 = bo * 1024
                        pss = alloc_round_psums()
                        for pair in range(NP8):
                            for ti, (xa, wb) in enumerate(TERMS):
                                xt = get_xe(bo, pair, xa)
                                wtile = wt[(w, wb)]
                                for c in range(4):
                                    for half in range(2):
                                        nc.tensor.matmul(
                                            pss[c * 2 + half],
                                            lhsT=wtile[:, pair, :,
                                                       c * P:(c + 1) * P],
                                            rhs=xt[:, :,
                                                   half * 512:(half + 1) * 512],
                                            perf_mode=DR,
                                            start=(pair == 0 and ti == 0),
                                            stop=(pair == NP8 - 1 and ti == 2),
                                        )
                        for c in range(4):
                            for half in range(2):
                                d_ = dst[:, c,
                                         t0 + half * 512:t0 + (half + 1) * 512]
                                if (c * 2 + half) % 2 == 0:
                                    nc.scalar.copy(d_, pss[c * 2 + half])
                                else:
                                    nc.vector.tensor_copy(d_, pss[c * 2 + half])

                    def emit_v_round(bo):
                        psv = alloc_round_psums()
                        for pair in range(NP8):
                            for ti, (xa, wb) in enumerate(TERMS):
                                xt = get_xe(bo, pair, xa)
                                wtile = wt[("v", wb)]
                                for tj in range(8):
                                    nc.tensor.matmul(
                                        psv[tj],
                                        lhsT=xt[:, :, tj * P:(tj + 1) * P],
                                        rhs=wtile[:, pair, :, :],
                                        perf_mode=DR,
                                        start=(pair == 0 and ti == 0),
                                        stop=(pair == NP8 - 1 and ti == 2),
                                    )
                        # split the copies of the last A round between
                        # DVE and ACT so the trailing drain that gates
                        # phase B's first PSUM allocations halves
                        for tj in range(8):
                            dst = vsb[:, bo * 8 + tj, :]
                            if tj % 2 == 0:
                                nc.scalar.copy(dst, psv[tj])
                            else:
                                nc.vector.tensor_copy(dst, psv[tj])

                    emit_qk_round(0, "q", qt)
                    emit_qk_round(0, "k", kt)
                    emit_v_round(0)
                    emit_qk_round(1, "k", kt)
                    emit_v_round(1)

                # --- B(tb0) with the b1-Q round injected 3 mm per s ---
                # sub-round h covers qt chunk c==h (heads line up with the
                # consumer B(tb1, h)); s-iters 0-7 accumulate half 0,
                # s-iters 8-15 half 1.
                for h in range(NHC):
                    state = {}

                    def q_hook(s, h=h, state=state):
                        half = s // 8
                        j = (s % 8) * 3           # 24 mms per half
                        if j == 0:
                            state[half] = pb_ps.tile(
                                [P, 512], f32, name="pqs", tag="pqs",
                                bufs=2)
                        pq = state[half]
                        for k in range(3):
                            idx = j + k
                            pair, ti = divmod(idx, 3)
                            xa, wb = TERMS[ti]
                            xt = get_xe(1, pair, xa)
                            nc.tensor.matmul(
                                pq,
                                lhsT=wt[("q", wb)][:, pair, :,
                                                   h * P:(h + 1) * P],
                                rhs=xt[:, :, half * 512:(half + 1) * 512],
                                perf_mode=DR,
                                start=(idx == 0), stop=(idx == 23),
                            )
                        if s % 8 == 7:
                            nc.vector.tensor_copy(
                                qt[:, h,
                                   1024 + half * 512:1024 + (half + 1) * 512],
                                pq)

                    emit_B(0, h, s_hook=q_hook)

            with tc.tile_pool(name="pd", bufs=1) as pd_pool:
                for h in range(NHC):
                    emit_B(1, h)
                    emit_D(0, 2 * h, pd_pool)
                    emit_D(0, 2 * h + 1, pd_pool)
                emit_D1_all(pd_pool)

    nc.compile()
    return nc


def _get_nc():
    global _NC_CACHE
    if _NC_CACHE is None:
        _NC_CACHE = _build_nc()
    return _NC_CACHE


def _split8(a, s):
    """a*s ~= hi + lo, both e4m3 at a common scale; returns fp8 arrays."""
    import ml_dtypes
    f8 = ml_dtypes.float8_e4m3fn
    a = np.asarray(a, np.float32) * s
    amax = float(np.abs(a).max())
    assert amax < 200.0, f"fp8 range: scaled absmax {amax}"
    hi = a.astype(f8)
    lo = (a - hi.astype(np.float32)).astype(f8)
    return hi, lo


def _interleave(a):
    """[E, M] -> [NP8, 128, 2, M] with e = pair*256 + i*128 + p."""
    M = a.shape[1]
    return np.ascontiguousarray(
        a.reshape(NP8, 2, P, M).transpose(0, 2, 1, 3))


def _interleave_w(a):
    """[E, M] -> [128, NP8, 2, M] with e = pair*256 + i*128 + p."""
    M = a.shape[1]
    return np.ascontiguousarray(
        a.reshape(NP8, 2, P, M).transpose(2, 0, 1, 3))


def _shard_inputs(x, W_attn, W_proj, lambda_q1, lambda_k1,
                  lambda_q2, lambda_k2):
    import ml_dtypes
    bf16 = ml_dtypes.bfloat16
    x = np.asarray(x, np.float32)
    W_attn = np.asarray(W_attn, np.float32)
    W_proj = np.asarray(W_proj, np.float32)
    lam = float(np.exp(np.dot(np.asarray(lambda_q1, np.float32),
                              np.asarray(lambda_k1, np.float32)))
                - np.exp(np.dot(np.asarray(lambda_q2, np.float32),
                                np.asarray(lambda_k2, np.float32)))
                + LAMBDA_INIT)
    Cb = E // 2  # 1024: q1/k1/q2/k2 block width in W_attn

    # x splits are shared by the 4 cores of each batch element
    xsplit = []
    for b in range(B):
        xh, xl = _split8(np.ascontiguousarray(x[b].T), SX)
        xsplit.append((_interleave(xh), _interleave(xl)))

    in_maps = []
    for c in range(N_CORES):
        b, hg = divmod(c, 4)
        heads = [4 * hg + j for j in range(NHC)]
        wq_c = np.empty((E, CQ), np.float32)
        wk_c = np.empty((E, CQ), np.float32)
        wv_c = np.empty((E, CQ), np.float32)
        wp_c = np.empty((CQ, E), np.float32)
        for j, h in enumerate(heads):
            wq_c[:, j * P:j * P + HD] = W_attn[:, h * HD:(h + 1) * HD] * SCALE
            wq_c[:, j * P + HD:(j + 1) * P] = (
                W_attn[:, 2 * Cb + h * HD:2 * Cb + (h + 1) * HD]
                * (-lam * SCALE))
            wk_c[:, j * P:j * P + HD] = W_attn[:, Cb + h * HD:Cb + (h + 1) * HD]
            wk_c[:, j * P + HD:(j + 1) * P] = (
                W_attn[:, 3 * Cb + h * HD:3 * Cb + (h + 1) * HD])
            wv_c[:, j * P:(j + 1) * P] = (
                W_attn[:, 4 * Cb + h * DV:4 * Cb + (h + 1) * DV])
            wp_c[j * P:(j + 1) * P, :] = (
                W_proj[h * DV:(h + 1) * DV, :]
                * ((1.0 - LAMBDA_INIT) / (SX * SWV)))
        wqh, wql = _split8(wq_c, SWQ)
        wkh, wkl = _split8(wk_c, SWK)
        wvh, wvl = _split8(wv_c, SWV)
        wph, wpl = _split8(wp_c, SWP)

        def _ilv_p(a):
            # [CQ, E] -> [P, 2jp, 2i, E] with dv = jp*256 + i*128 + p
            return np.ascontiguousarray(
                a.reshape(2, 2, P, E).transpose(2, 0, 1, 3))

        xh8, xl8 = xsplit[b]
        in_maps.append({
            "xh": xh8, "xl": xl8,
            "wqh": _interleave_w(wqh), "wql": _interleave_w(wql),
            "wkh": _interleave_w(wkh), "wkl": _interleave_w(wkl),
            "wvh": _interleave_w(wvh), "wvl": _interleave_w(wvl),
            "wph": _ilv_p(wph), "wpl": _ilv_p(wpl),
        })
    return in_maps


def _run(inputs, trace=False):
    from concourse.bass_utils import run_bass_kernel_spmd
    nc = _get_nc()
    in_maps = _shard_inputs(**inputs)
    res = run_bass_kernel_spmd(nc, in_maps, list(range(N_CORES)),
                               trace=trace)
    out = np.zeros((B, T, E), np.float32)
    for c in range(N_CORES):
        out[c // 4] += res.results[c]["out"].astype(np.float32)
    out *= S_OUT
    return out, res


def kernel(x, W_attn, W_proj, lambda_q1, lambda_k1, lambda_q2, lambda_k2):
    out, _ = _run(dict(x=x, W_attn=W_attn, W_proj=W_proj,
                       lambda_q1=lambda_q1, lambda_k1=lambda_k1,
                       lambda_q2=lambda_q2, lambda_k2=lambda_k2))
    return out


# revision 16
# speedup vs baseline: 1.0710x; 1.0187x over previous
"""MultiHeadDiffAttention Trainium2 kernel, v3.

Strategy (8 NeuronCores, SPMD), same sharding as baseline:
  core c handles b = c//4, heads 4*(c%4)..4*(c%4)+3.

Perf structure (CoreSim-modeled, HW-verified numerics):
  1. QKV projection runs as error-compensated split-fp8 (e4m3 hi+lo with a
     shared power-of-2 scale, 3 of 4 cross terms) using DoubleRow matmuls:
     each instruction contracts 256 rows at 0.5 cycles/row, so the
     projection costs 0.75x the fp32r schedule at ~bf16 accuracy.
     x and W are split/interleaved host-side; all descale factors fold into
     the exp() scale and the (host-prepared) W_proj.
  2. The softmax denominator comes from a bf16 DVE accumulation of the exp
     tiles plus one Pool partition_all_reduce - no PE ones-matmuls.
  3. exp() runs 1024-wide (2 PSUM banks per logits tile) to halve ACT's
     per-instruction access overhead; q/k are stored fp16 (smaller scales)
     so the large-logit heads keep ~1e-3 score accuracy.
  4. Software pipelining: the t-block-1 Q projection is injected 3 matmuls
     per s-iteration into the first attention block (which is otherwise
     ACT-bound), and the output projection of block 0 is interleaved with
     the second attention block, so PE never drains between phases.
"""

import math

import numpy as np

B, T, E = 2, 2048, 2048
N_HEAD = 16
HD = 64                       # per-component head dim (q1/k1/q2/k2)
DV = 128                      # v head dim
SCALE = HD ** -0.5
LAMBDA_INIT = 0.8 - 0.6 * math.exp(-0.3 * (1 - 1))
P = 128
NHC = 4                       # heads per core
CQ = NHC * DV                 # 512: per-core q'/k'/v width
N_CORES = 8
NE = E // P                   # 16 contraction chunks
NP8 = NE // 2                 # 8 e-pair chunks for DoubleRow
NS = T // P                   # 16 s chunks

# power-of-2 quantization scales (chosen for the spec's randn*0.02 weight /
# randn activation distributions with >=2x headroom in e4m3 and fp16)
SX = 16.0
SWQ = 512.0
SWK = 256.0
SWV = 1024.0
S_EXP = 1.0 / (SX * SX * SWQ * SWK)   # folds all q/k descales into exp()
S_OT = 1.0 / 512.0                     # ot -> fp8 hi/lo scale
SWP = float(2 ** 22)                  # W_proj fp8 scale (after v-descale fold)
S_OUT = 1.0 / (S_OT * SWP)            # final output descale

TERMS = (("h", "h"), ("h", "l"), ("l", "h"))   # (x side, w side) fp8 terms

_NC_CACHE = None


def _build_nc():
    import concourse.mybir as mybir
    import concourse.tile as tile
    from concourse import bacc, bass_isa

    f32 = mybir.dt.float32
    bf16 = mybir.dt.bfloat16
    f16 = mybir.dt.float16
    fp8 = mybir.dt.float8e4
    DR = mybir.MatmulPerfMode.DoubleRow
    EXP = mybir.ActivationFunctionType.Exp
    COPY = mybir.ActivationFunctionType.Copy

    nc = bacc.Bacc("TRN2", target_bir_lowering=False, debug=False,
                   num_devices=N_CORES)
    xd = {a: nc.dram_tensor(f"x{a}", [NP8, P, 2, T], fp8,
                            kind="ExternalInput").ap() for a in "hl"}
    wd = {(w, a): nc.dram_tensor(f"w{w}{a}", [P, NP8, 2, CQ], fp8,
                                 kind="ExternalInput").ap()
          for w in "qkv" for a in "hl"}
    wpd = {a: nc.dram_tensor(f"wp{a}", [P, 2, 2, E], fp8,
                             kind="ExternalInput").ap() for a in "hl"}
    out = nc.dram_tensor("out", [T, E], bf16, kind="ExternalOutput").ap()

    with tile.TileContext(nc) as tc:
        with (
            tc.tile_pool(name="res", bufs=1) as res,
            tc.tile_pool(name="pb", bufs=1) as pb,
            tc.tile_pool(name="pb_ps", bufs=1, space="PSUM") as pb_ps,
        ):
            qt = res.tile([P, NHC, T], f16, name="qt")     # q' * SX*SWQ
            kt = res.tile([P, NHC, T], f16, name="kt")     # k' * SX*SWK
            vsb = res.tile([P, NS, CQ], bf16, name="vsb")  # v * SX*SWV
            ot = res.tile([P, NHC, T], bf16, name="ot")    # O^T per head
            oth = res.tile([P, NHC, T], fp8, name="oth")   # ot*S_OT hi
            otl = res.tile([P, NHC, T], fp8, name="otl")   # ot*S_OT lo
            wpt8 = {}
            for a in "hl":
                wpt8[a] = res.tile([P, 2, 2, E], fp8, name=f"wpt{a}")
                nc.scalar.dma_start(wpt8[a], wpd[a])

            # ---------- phase helpers ---------------------------------

            def emit_B(tb, h, s_hook=None):
                """Attention for (t-block tb, head h); s_hook(s) may inject
                extra PE work after each logits matmul pair."""
                t0 = tb * 1024
                pso = [
                    pb_ps.tile([P, 512], f32, name=f"pso{i}",
                               tag=f"pso{i}", bufs=1)
                    for i in range(2)
                ]
                acc2 = pb.tile([P, 1024], bf16, name="acc", tag="acc",
                               bufs=2)
                for s in range(NS):
                    psa2 = pb_ps.tile([P, 1024], f32, name="psa",
                                      tag="psa", bufs=2)
                    for half in range(2):
                        nc.tensor.matmul(
                            psa2[:, half * 512:(half + 1) * 512],
                            lhsT=kt[:, h, s * P:(s + 1) * P],
                            rhs=qt[:, h,
                                   t0 + half * 512:t0 + (half + 1) * 512],
                            start=True, stop=True,
                        )
                    if s_hook is not None:
                        s_hook(s)
                    et2 = pb.tile([P, 1024], bf16, name="et", tag="et",
                                  bufs=4)
                    nc.scalar.activation(et2, psa2, EXP, scale=S_EXP)
                    for half in range(2):
                        nc.tensor.matmul(
                            pso[half],
                            lhsT=vsb[:, s, h * P:(h + 1) * P],
                            rhs=et2[:, half * 512:(half + 1) * 512],
                            start=(s == 0), stop=(s == NS - 1),
                        )
                    if s == 0:
                        nc.vector.tensor_copy(acc2, et2)
                    else:
                        nc.vector.tensor_add(acc2, acc2, et2)
                zs = pb.tile([P, 1024], f32, name="zs", tag="zs", bufs=1)
                rb = pb.tile([P, 1024], f32, name="rb", tag="rb", bufs=2)
                last = (tb == 1 and h == NHC - 1)
                for half in ((0, 1) if last else (None,)):
                    # the last block pipelines the whole Z/normalize/split
                    # chain per 512-half so D1's first t-chunks (which only
                    # read half 0) start ~3us sooner
                    if half is None:
                        hs = slice(0, 1024)
                        ts_ = slice(t0, t0 + 1024)
                        psos = pso
                    else:
                        hs = slice(half * 512, (half + 1) * 512)
                        ts_ = slice(t0 + half * 512, t0 + (half + 1) * 512)
                        psos = [pso[half]]
                    nc.gpsimd.partition_all_reduce(
                        zs[:, hs], acc2[:, hs], channels=P,
                        reduce_op=bass_isa.ReduceOp.add)
                    nc.vector.reciprocal(rb[:, hs], zs[:, hs])
                    for i, ps in enumerate(psos):
                        w0 = (hs.start if half is not None else i * 512)
                        nc.vector.tensor_mul(
                            ot[:, h, t0 + w0:t0 + w0 + 512],
                            ps, rb[:, w0:w0 + 512])
                    osl = (slice(None), h, ts_)
                    nc.scalar.activation(oth[osl], ot[osl], COPY,
                                         scale=S_OT)
                    nc.vector.scalar_tensor_tensor(
                        otl[osl], ot[osl], S_OT, oth[osl],
                        mybir.AluOpType.mult, mybir.AluOpType.subtract)

            D_TERMS = ((oth, "h"), (oth, "l"), (otl, "h"))

            def d_mms(psd, tg, eo, jp):
                for ti, (osrc, wb) in enumerate(D_TERMS):
                    nc.tensor.matmul(
                        psd,
                        lhsT=osrc[:, 2 * jp:2 * jp + 2,
                                  tg * P:(tg + 1) * P],
                        rhs=wpt8[wb][:, jp, :, eo * 512:(eo + 1) * 512],
                        perf_mode=DR,
                        start=(jp == 0 and ti == 0),
                        stop=(jp == 1 and ti == 2),
                    )

            def emit_D(tb, tj, pd_pool):
                tg = tb * 8 + tj
                osb4 = pd_pool.tile([P, 4, 512], bf16, name="osb4",
                                    tag="osb4", bufs=4)
                for eo in range(4):
                    psd = pb_ps.tile([P, 512], f32, name="psd",
                                     tag="pqs", bufs=2)
                    d_mms(psd, tg, eo, 0)
                    d_mms(psd, tg, eo, 1)
                    # descale happens host-side; split staging copies so
                    # neither ACT (exp stream) nor DVE saturates
                    if eo == 1:
                        nc.scalar.copy(osb4[:, eo, :], psd)
                    else:
                        nc.vector.tensor_copy(osb4[:, eo, :], psd)
                dma_eng = nc.sync if tj % 2 == 0 else nc.gpsimd
                dma_eng.dma_start(out[tg * P:(tg + 1) * P, :], osb4)

            def emit_D1_all(pd_pool):
                # tail block: rotate psum through every free tag (the
                # attention tags are done) and run jp0 of the next groups
                # ahead of jp1 (which waits on the last head's normalize)
                slots = (("pqs", 2), ("pqs", 2), ("pso0", 1), ("pso1", 1),
                         ("psa", 2), ("psa", 2))
                groups = [(8 + tj, eo) for tj in range(8)
                          for eo in range(4)]
                DEPTH = 5
                osb = {}
                pend = []

                def finish(psd, tg, eo, gi):
                    d_mms(psd, tg, eo, 1)
                    tj = tg - 8
                    if tj not in osb:
                        osb[tj] = pd_pool.tile([P, 4, 512], bf16,
                                               name="osb4", tag="osb4",
                                               bufs=4)
                    if gi % 2:
                        nc.scalar.copy(osb[tj][:, eo, :], psd)
                    else:
                        nc.vector.tensor_copy(osb[tj][:, eo, :], psd)
                    if eo == 3:
                        ob = osb.pop(tj)
                        if tj >= 6:
                            for k, eng in enumerate(
                                    (nc.sync, nc.scalar, nc.gpsimd,
                                     nc.sync)):
                                eng.dma_start(
                                    out[tg * P:(tg + 1) * P,
                                        k * 512:(k + 1) * 512],
                                    ob[:, k, :])
                        else:
                            eng = (nc.sync, nc.scalar, nc.gpsimd)[tj % 3]
                            eng.dma_start(out[tg * P:(tg + 1) * P, :], ob)

                for gi, (tg, eo) in enumerate(groups):
                    tag, bufs = slots[gi % len(slots)]
                    psd = pb_ps.tile([P, 512], f32, name="psd1",
                                     tag=tag, bufs=bufs)
                    d_mms(psd, tg, eo, 0)
                    pend.append((psd, tg, eo, gi))
                    if len(pend) >= DEPTH:
                        finish(*pend.pop(0))
                for args in pend:
                    finish(*args)

            # ---------- Phase A prefix + pipelined B/D -------------------
            with (
                tc.tile_pool(name="pa_w", bufs=1) as pa_w,
                tc.tile_pool(name="pa_x", bufs=1) as pa_x,
            ):
                wt = {}
                for w in "qkv":
                    for a in "hl":
                        wt[(w, a)] = pa_w.tile([P, NP8, 2, CQ], fp8,
                                               name=f"w{w}{a}",
                                               tag=f"w{w}{a}", bufs=1)
                for a in "hl":
                    nc.sync.dma_start(wt[("q", a)][:, 0:1],
                                      wd[("q", a)][:, 0:1])
                for hp, lo, hi in ((0, 1, 4), (1, 4, 8)):
                    for w in "qkv":
                        for a in "hl":
                            if w == "q":
                                nc.sync.dma_start(wt[(w, a)][:, lo:hi],
                                                  wd[(w, a)][:, lo:hi])
                            else:
                                nc.sync.dma_start(
                                    wt[(w, a)][:, hp * 4:(hp + 1) * 4],
                                    wd[(w, a)][:, hp * 4:(hp + 1) * 4])

                xe_blocks = [{}, {}]

                def get_xe(bo, pair, a):
                    # tags are shared between the two t-blocks (16 tags,
                    # bufs=1): block 1's DMA starts as soon as block 0's
                    # last reader of that tag is done
                    xe = xe_blocks[bo]
                    if (pair, a) not in xe:
                        tl = pa_x.tile([P, 2, 1024], fp8,
                                       name=f"xe{bo}{pair}{a}",
                                       tag=f"xe{pair}{a}", bufs=1)
                        nc.gpsimd.dma_start(
                            tl, xd[a][pair, :, :, bo * 1024:(bo + 1) * 1024])
                        xe[(pair, a)] = tl
                    return xe[(pair, a)]

                # A-round psum tiles borrow phase B's tag slots (2x[P,1024]
                # "psa" + pso0 + pso1 + 2x[P,512] "pqs" = exactly 8 banks);
                # the returned list holds 8 [P,512] views indexed c*2+half.
                def alloc_round_psums():
                    pa0 = pb_ps.tile([P, 1024], f32, name="pssA",
                                     tag="psa", bufs=2)
                    pa1 = pb_ps.tile([P, 1024], f32, name="pssB",
                                     tag="psa", bufs=2)
                    o0 = pb_ps.tile([P, 512], f32, name="pssC",
                                    tag="pso0", bufs=1)
                    o1 = pb_ps.tile([P, 512], f32, name="pssD",
                                    tag="pso1", bufs=1)
                    q0 = pb_ps.tile([P, 512], f32, name="pssE",
                                    tag="pqs", bufs=2)
                    q1 = pb_ps.tile([P, 512], f32, name="pssF",
                                    tag="pqs", bufs=2)
                    return [pa0[:, 0:512], pa0[:, 512:1024],
                            pa1[:, 0:512], pa1[:, 512:1024],
                            o0, o1, q0, q1]

                if True:
                    # --- A prefix: b0 [Q K V], b1 [K V] ---
                    def emit_qk_round(bo, w, dst):
                        t0 = bo * 1024
                        pss = alloc_round_psums()
                        def qk_mm(pair, ti, c, half):
                            xa, wb = TERMS[ti]
                            nc.tensor.matmul(
                                pss[c * 2 + half],
                                lhsT=wt[(w, wb)][:, pair, :,
                                                 c * P:(c + 1) * P],
                                rhs=get_xe(bo, pair, xa)[
                                    :, :, half * 512:(half + 1) * 512],
                                perf_mode=DR,
                                start=(pair == 0 and ti == 0),
                                stop=(pair == NP8 - 1 and ti == 2),
                            )

                        for pair in range(NP8 - 1):
                            for ti in range(3):
                                for c in range(4):
                                    for half in range(2):
                                        qk_mm(pair, ti, c, half)
                        # last pair: finish view (c,half) completely before
                        # moving on, so its psum copy overlaps the round
                        # tail instead of stalling the next round
                        for c in range(4):
                            for half in range(2):
                                for ti in range(3):
                                    qk_mm(NP8 - 1, ti, c, half)
                        for c in range(4):
                            for half in range(2):
                                d_ = dst[:, c,
                                         t0 + half * 512:t0 + (half + 1) * 512]
                                if (c * 2 + half) % 2 == 0:
                                    nc.scalar.copy(d_, pss[c * 2 + half])
                                else:
                                    nc.vector.tensor_copy(d_, pss[c * 2 + half])

                    def emit_v_round(bo):
                        psv = alloc_round_psums()
                        def v_mm(pair, ti, tj):
                            xa, wb = TERMS[ti]
                            nc.tensor.matmul(
                                psv[tj],
                                lhsT=get_xe(bo, pair, xa)[
                                    :, :, tj * P:(tj + 1) * P],
                                rhs=wt[("v", wb)][:, pair, :, :],
                                perf_mode=DR,
                                start=(pair == 0 and ti == 0),
                                stop=(pair == NP8 - 1 and ti == 2),
                            )

                        for pair in range(NP8 - 1):
                            for ti in range(3):
                                for tj in range(8):
                                    v_mm(pair, ti, tj)
                        for tj in range(8):
                            for ti in range(3):
                                v_mm(NP8 - 1, ti, tj)
                        # split the copies of the last A round between
                        # DVE and ACT so the trailing drain that gates
                        # phase B's first PSUM allocations halves
                        for tj in range(8):
                            dst = vsb[:, bo * 8 + tj, :]
                            if tj % 2 == 0:
                                nc.scalar.copy(dst, psv[tj])
                            else:
                                nc.vector.tensor_copy(dst, psv[tj])

                    emit_qk_round(0, "q", qt)
                    emit_qk_round(0, "k", kt)
                    emit_v_round(0)
                    emit_qk_round(1, "k", kt)
                    emit_v_round(1)

                # --- B(tb0) with the b1-Q round injected 3 mm per s ---
                # sub-round h covers qt chunk c==h (heads line up with the
                # consumer B(tb1, h)); s-iters 0-7 accumulate half 0,
                # s-iters 8-15 half 1.
                for h in range(NHC):
                    state = {}

                    def q_hook(s, h=h, state=state):
                        half = s // 8
                        j = (s % 8) * 3           # 24 mms per half
                        if j == 0:
                            state[half] = pb_ps.tile(
                                [P, 512], f32, name="pqs", tag="pqs",
                                bufs=2)
                        pq = state[half]
                        for k in range(3):
                            idx = j + k
                            pair, ti = divmod(idx, 3)
                            xa, wb = TERMS[ti]
                            xt = get_xe(1, pair, xa)
                            nc.tensor.matmul(
                                pq,
                                lhsT=wt[("q", wb)][:, pair, :,
                                                   h * P:(h + 1) * P],
                                rhs=xt[:, :, half * 512:(half + 1) * 512],
                                perf_mode=DR,
                                start=(idx == 0), stop=(idx == 23),
                            )
                        if s % 8 == 7:
                            nc.vector.tensor_copy(
                                qt[:, h,
                                   1024 + half * 512:1024 + (half + 1) * 512],
                                pq)

                    emit_B(0, h, s_hook=q_hook)

            with tc.tile_pool(name="pd", bufs=1) as pd_pool:
                for h in range(NHC):
                    emit_B(1, h)
                    emit_D(0, 2 * h, pd_pool)
                    emit_D(0, 2 * h + 1, pd_pool)
                emit_D1_all(pd_pool)

    nc.compile()
    return nc


def _get_nc():
    global _NC_CACHE
    if _NC_CACHE is None:
        _NC_CACHE = _build_nc()
    return _NC_CACHE


def _split8(a, s):
    """a*s ~= hi + lo, both e4m3 at a common scale; returns fp8 arrays."""
    import ml_dtypes
    f8 = ml_dtypes.float8_e4m3fn
    a = np.asarray(a, np.float32) * s
    amax = float(np.abs(a).max())
    assert amax < 200.0, f"fp8 range: scaled absmax {amax}"
    hi = a.astype(f8)
    lo = (a - hi.astype(np.float32)).astype(f8)
    return hi, lo


def _interleave(a):
    """[E, M] -> [NP8, 128, 2, M] with e = pair*256 + i*128 + p."""
    M = a.shape[1]
    return np.ascontiguousarray(
        a.reshape(NP8, 2, P, M).transpose(0, 2, 1, 3))


def _interleave_w(a):
    """[E, M] -> [128, NP8, 2, M] with e = pair*256 + i*128 + p."""
    M = a.shape[1]
    return np.ascontiguousarray(
        a.reshape(NP8, 2, P, M).transpose(2, 0, 1, 3))


def _shard_inputs(x, W_attn, W_proj, lambda_q1, lambda_k1,
                  lambda_q2, lambda_k2):
    import ml_dtypes
    bf16 = ml_dtypes.bfloat16
    x = np.asarray(x, np.float32)
    W_attn = np.asarray(W_attn, np.float32)
    W_proj = np.asarray(W_proj, np.float32)
    lam = float(np.exp(np.dot(np.asarray(lambda_q1, np.float32),
                              np.asarray(lambda_k1, np.float32)))
                - np.exp(np.dot(np.asarray(lambda_q2, np.float32),
                                np.asarray(lambda_k2, np.float32)))
                + LAMBDA_INIT)
    Cb = E // 2  # 1024: q1/k1/q2/k2 block width in W_attn

    # x splits are shared by the 4 cores of each batch element
    xsplit = []
    for b in range(B):
        xh, xl = _split8(np.ascontiguousarray(x[b].T), SX)
        xsplit.append((_interleave(xh), _interleave(xl)))

    in_maps = []
    for c in range(N_CORES):
        b, hg = divmod(c, 4)
        heads = [4 * hg + j for j in range(NHC)]
        wq_c = np.empty((E, CQ), np.float32)
        wk_c = np.empty((E, CQ), np.float32)
        wv_c = np.empty((E, CQ), np.float32)
        wp_c = np.empty((CQ, E), np.float32)
        for j, h in enumerate(heads):
            wq_c[:, j * P:j * P + HD] = W_attn[:, h * HD:(h + 1) * HD] * SCALE
            wq_c[:, j * P + HD:(j + 1) * P] = (
                W_attn[:, 2 * Cb + h * HD:2 * Cb + (h + 1) * HD]
                * (-lam * SCALE))
            wk_c[:, j * P:j * P + HD] = W_attn[:, Cb + h * HD:Cb + (h + 1) * HD]
            wk_c[:, j * P + HD:(j + 1) * P] = (
                W_attn[:, 3 * Cb + h * HD:3 * Cb + (h + 1) * HD])
            wv_c[:, j * P:(j + 1) * P] = (
                W_attn[:, 4 * Cb + h * DV:4 * Cb + (h + 1) * DV])
            wp_c[j * P:(j + 1) * P, :] = (
                W_proj[h * DV:(h + 1) * DV, :]
                * ((1.0 - LAMBDA_INIT) / (SX * SWV)))
        wqh, wql = _split8(wq_c, SWQ)
        wkh, wkl = _split8(wk_c, SWK)
        wvh, wvl = _split8(wv_c, SWV)
        wph, wpl = _split8(wp_c, SWP)

        def _ilv_p(a):
            # [CQ, E] -> [P, 2jp, 2i, E] with dv = jp*256 + i*128 + p
            return np.ascontiguousarray(
                a.reshape(2, 2, P, E).transpose(2, 0, 1, 3))

        xh8, xl8 = xsplit[b]
        in_maps.append({
            "xh": xh8, "xl": xl8,
            "wqh": _interleave_w(wqh), "wql": _interleave_w(wql),
            "wkh": _interleave_w(wkh), "wkl": _interleave_w(wkl),
            "wvh": _interleave_w(wvh), "wvl": _interleave_w(wvl),
            "wph": _ilv_p(wph), "wpl": _ilv_p(wpl),
        })
    return in_maps


def _run(inputs, trace=False):
    from concourse.bass_utils import run_bass_kernel_spmd
    nc = _get_nc()
    in_maps = _shard_inputs(**inputs)
    res = run_bass_kernel_spmd(nc, in_maps, list(range(N_CORES)),
                               trace=trace)
    out = np.zeros((B, T, E), np.float32)
    for c in range(N_CORES):
        out[c // 4] += res.results[c]["out"].astype(np.float32)
    out *= S_OUT
    return out, res


def kernel(x, W_attn, W_proj, lambda_q1, lambda_k1, lambda_q2, lambda_k2):
    out, _ = _run(dict(x=x, W_attn=W_attn, W_proj=W_proj,
                       lambda_q1=lambda_q1, lambda_k1=lambda_k1,
                       lambda_q2=lambda_q2, lambda_k2=lambda_k2))
    return out


# revision 22
# speedup vs baseline: 1.0775x; 1.0061x over previous
"""MultiHeadDiffAttention Trainium2 kernel, v3.

Strategy (8 NeuronCores, SPMD), same sharding as baseline:
  core c handles b = c//4, heads 4*(c%4)..4*(c%4)+3.

Perf structure (CoreSim-modeled, HW-verified numerics):
  1. QKV projection runs as error-compensated split-fp8 (e4m3 hi+lo with a
     shared power-of-2 scale, 3 of 4 cross terms) using DoubleRow matmuls:
     each instruction contracts 256 rows at 0.5 cycles/row, so the
     projection costs 0.75x the fp32r schedule at ~bf16 accuracy.
     x and W are split/interleaved host-side; all descale factors fold into
     the exp() scale and the (host-prepared) W_proj.
  2. The softmax denominator comes from a bf16 DVE accumulation of the exp
     tiles plus one Pool partition_all_reduce - no PE ones-matmuls.
  3. exp() runs 1024-wide (2 PSUM banks per logits tile) to halve ACT's
     per-instruction access overhead; q/k are stored fp16 (smaller scales)
     so the large-logit heads keep ~1e-3 score accuracy.
  4. Software pipelining: the t-block-1 Q projection is injected 3 matmuls
     per s-iteration into the first attention block (which is otherwise
     ACT-bound), and the output projection of block 0 is interleaved with
     the second attention block, so PE never drains between phases.
"""

import math

import numpy as np

B, T, E = 2, 2048, 2048
N_HEAD = 16
HD = 64                       # per-component head dim (q1/k1/q2/k2)
DV = 128                      # v head dim
SCALE = HD ** -0.5
LAMBDA_INIT = 0.8 - 0.6 * math.exp(-0.3 * (1 - 1))
P = 128
NHC = 4                       # heads per core
CQ = NHC * DV                 # 512: per-core q'/k'/v width
N_CORES = 8
NE = E // P                   # 16 contraction chunks
NP8 = NE // 2                 # 8 e-pair chunks for DoubleRow
NS = T // P                   # 16 s chunks

# power-of-2 quantization scales (chosen for the spec's randn*0.02 weight /
# randn activation distributions with >=2x headroom in e4m3 and fp16)
SX = 16.0
SWQ = 512.0
SWK = 256.0
SWV = 1024.0
S_EXP = 1.0 / (SX * SX * SWQ * SWK)   # folds all q/k descales into exp()
S_OT = 1.0 / 512.0                     # ot -> fp8 hi/lo scale
SWP = float(2 ** 22)                  # W_proj fp8 scale (after v-descale fold)
S_OUT = 1.0 / (S_OT * SWP)            # final output descale

TERMS = (("h", "h"), ("h", "l"), ("l", "h"))   # (x side, w side) fp8 terms

_NC_CACHE = None


def _build_nc():
    import concourse.mybir as mybir
    import concourse.tile as tile
    from concourse import bacc, bass_isa

    f32 = mybir.dt.float32
    bf16 = mybir.dt.bfloat16
    f16 = mybir.dt.float16
    fp8 = mybir.dt.float8e4
    DR = mybir.MatmulPerfMode.DoubleRow
    EXP = mybir.ActivationFunctionType.Exp
    COPY = mybir.ActivationFunctionType.Copy

    nc = bacc.Bacc("TRN2", target_bir_lowering=False, debug=False,
                   num_devices=N_CORES)
    xd = {a: nc.dram_tensor(f"x{a}", [NP8, P, 2, T], fp8,
                            kind="ExternalInput").ap() for a in "hl"}
    wd = {(w, a): nc.dram_tensor(f"w{w}{a}", [P, NP8, 2, CQ], fp8,
                                 kind="ExternalInput").ap()
          for w in "qkv" for a in "hl"}
    wpd = {a: nc.dram_tensor(f"wp{a}", [P, 2, 2, E], fp8,
                             kind="ExternalInput").ap() for a in "hl"}
    out = nc.dram_tensor("out", [T, E], bf16, kind="ExternalOutput").ap()

    with tile.TileContext(nc) as tc:
        with (
            tc.tile_pool(name="res", bufs=1) as res,
            tc.tile_pool(name="pb", bufs=1) as pb,
            tc.tile_pool(name="pb_ps", bufs=1, space="PSUM") as pb_ps,
        ):
            qt = res.tile([P, NHC, T], f16, name="qt")     # q' * SX*SWQ
            kt = res.tile([P, NHC, T], f16, name="kt")     # k' * SX*SWK
            vsb = res.tile([P, NS, CQ], bf16, name="vsb")  # v * SX*SWV
            ot = res.tile([P, NHC, T], bf16, name="ot")    # O^T per head
            oth = res.tile([P, NHC, T], fp8, name="oth")   # ot*S_OT hi
            otl = res.tile([P, NHC, T], fp8, name="otl")   # ot*S_OT lo
            wpt8 = {}
            for a in "hl":
                wpt8[a] = res.tile([P, 2, 2, E], fp8, name=f"wpt{a}")
                nc.scalar.dma_start(wpt8[a], wpd[a])

            # ---------- phase helpers ---------------------------------

            def emit_B(tb, h, s_hook=None):
                """Attention for (t-block tb, head h); s_hook(s) may inject
                extra PE work after each logits matmul pair."""
                t0 = tb * 1024
                pso = [
                    pb_ps.tile([P, 512], f32, name=f"pso{i}",
                               tag=f"pso{i}", bufs=1)
                    for i in range(2)
                ]
                acc2 = pb.tile([P, 1024], bf16, name="acc", tag="acc",
                               bufs=2)
                for s in range(NS):
                    psa2 = pb_ps.tile([P, 1024], f32, name="psa",
                                      tag="psa", bufs=2)
                    for half in range(2):
                        nc.tensor.matmul(
                            psa2[:, half * 512:(half + 1) * 512],
                            lhsT=kt[:, h, s * P:(s + 1) * P],
                            rhs=qt[:, h,
                                   t0 + half * 512:t0 + (half + 1) * 512],
                            start=True, stop=True,
                        )
                    if s_hook is not None:
                        s_hook(s)
                    et2 = pb.tile([P, 1024], bf16, name="et", tag="et",
                                  bufs=4)
                    nc.scalar.activation(et2, psa2, EXP, scale=S_EXP)
                    for half in range(2):
                        nc.tensor.matmul(
                            pso[half],
                            lhsT=vsb[:, s, h * P:(h + 1) * P],
                            rhs=et2[:, half * 512:(half + 1) * 512],
                            start=(s == 0), stop=(s == NS - 1),
                        )
                    if s == 0:
                        nc.vector.tensor_copy(acc2, et2)
                    else:
                        nc.vector.tensor_add(acc2, acc2, et2)
                zs = pb.tile([P, 1024], f32, name="zs", tag="zs", bufs=1)
                rb = pb.tile([P, 1024], f32, name="rb", tag="rb", bufs=2)
                last = (tb == 1 and h == NHC - 1)
                for half in ((0, 1) if last else (None,)):
                    # the last block pipelines the whole Z/normalize/split
                    # chain per 512-half so D1's first t-chunks (which only
                    # read half 0) start ~3us sooner
                    if half is None:
                        hs = slice(0, 1024)
                        ts_ = slice(t0, t0 + 1024)
                        psos = pso
                    else:
                        hs = slice(half * 512, (half + 1) * 512)
                        ts_ = slice(t0 + half * 512, t0 + (half + 1) * 512)
                        psos = [pso[half]]
                    nc.gpsimd.partition_all_reduce(
                        zs[:, hs], acc2[:, hs], channels=P,
                        reduce_op=bass_isa.ReduceOp.add)
                    nc.vector.reciprocal(rb[:, hs], zs[:, hs])
                    for i, ps in enumerate(psos):
                        w0 = (hs.start if half is not None else i * 512)
                        nc.vector.tensor_mul(
                            ot[:, h, t0 + w0:t0 + w0 + 512],
                            ps, rb[:, w0:w0 + 512])
                    osl = (slice(None), h, ts_)
                    if tb == 0:
                        nc.vector.tensor_scalar_mul(oth[osl], ot[osl], S_OT)
                    else:
                        nc.scalar.activation(oth[osl], ot[osl], COPY,
                                             scale=S_OT)
                    nc.vector.scalar_tensor_tensor(
                        otl[osl], ot[osl], S_OT, oth[osl],
                        mybir.AluOpType.mult, mybir.AluOpType.subtract)

            D_TERMS = ((oth, "h"), (oth, "l"), (otl, "h"))

            def d_mms(psd, tg, eo, jp):
                for ti, (osrc, wb) in enumerate(D_TERMS):
                    nc.tensor.matmul(
                        psd,
                        lhsT=osrc[:, 2 * jp:2 * jp + 2,
                                  tg * P:(tg + 1) * P],
                        rhs=wpt8[wb][:, jp, :, eo * 512:(eo + 1) * 512],
                        perf_mode=DR,
                        start=(jp == 0 and ti == 0),
                        stop=(jp == 1 and ti == 2),
                    )

            def emit_D(tb, tj, pd_pool):
                tg = tb * 8 + tj
                osb4 = pd_pool.tile([P, 4, 512], bf16, name="osb4",
                                    tag="osb4", bufs=4)
                for eo in range(4):
                    psd = pb_ps.tile([P, 512], f32, name="psd",
                                     tag="pqs", bufs=2)
                    d_mms(psd, tg, eo, 0)
                    d_mms(psd, tg, eo, 1)
                    # descale happens host-side; split staging copies so
                    # neither ACT (exp stream) nor DVE saturates
                    if eo == 1:
                        nc.scalar.copy(osb4[:, eo, :], psd)
                    else:
                        nc.vector.tensor_copy(osb4[:, eo, :], psd)
                dma_eng = nc.sync if tj % 2 == 0 else nc.gpsimd
                dma_eng.dma_start(out[tg * P:(tg + 1) * P, :], osb4)

            def emit_D1_all(pd_pool):
                # tail block: rotate psum through every free tag (the
                # attention tags are done) and run jp0 of the next groups
                # ahead of jp1 (which waits on the last head's normalize)
                groups = [(8 + tj, eo) for tj in range(8)
                          for eo in range(4)]
                DEPTH = 6
                osb = {}
                pend = []
                psa_half = [None]

                def alloc_psd(gi):
                    kind = gi % 4
                    if kind in (0, 1):
                        return pb_ps.tile([P, 512], f32, name="psd1",
                                          tag="pqs", bufs=2)
                    if kind == 2:
                        return pb_ps.tile([P, 512], f32, name="psd2",
                                          tag=f"pso{(gi // 4) % 2}", bufs=1)
                    if psa_half[0] is None:
                        big = pb_ps.tile([P, 1024], f32, name="psd3",
                                         tag="psa", bufs=2)
                        psa_half[0] = big[:, 512:1024]
                        return big[:, 0:512]
                    v = psa_half[0]
                    psa_half[0] = None
                    return v

                def finish(psd, tg, eo, gi):
                    d_mms(psd, tg, eo, 1)
                    tj = tg - 8
                    if tj not in osb:
                        osb[tj] = pd_pool.tile([P, 4, 512], bf16,
                                               name="osb4", tag="osb4",
                                               bufs=4)
                    if gi % 2:
                        nc.scalar.copy(osb[tj][:, eo, :], psd)
                    else:
                        nc.vector.tensor_copy(osb[tj][:, eo, :], psd)
                    if eo == 3:
                        ob = osb.pop(tj)
                        if tj >= 6:
                            for k, eng in enumerate(
                                    (nc.sync, nc.scalar, nc.gpsimd,
                                     nc.sync)):
                                eng.dma_start(
                                    out[tg * P:(tg + 1) * P,
                                        k * 512:(k + 1) * 512],
                                    ob[:, k, :])
                        else:
                            eng = (nc.sync, nc.scalar, nc.gpsimd)[tj % 3]
                            eng.dma_start(out[tg * P:(tg + 1) * P, :], ob)

                for gi, (tg, eo) in enumerate(groups):
                    psd = alloc_psd(gi)
                    d_mms(psd, tg, eo, 0)
                    pend.append((psd, tg, eo, gi))
                    if len(pend) >= DEPTH:
                        finish(*pend.pop(0))
                for args in pend:
                    finish(*args)

            # ---------- Phase A prefix + pipelined B/D -------------------
            with (
                tc.tile_pool(name="pa_w", bufs=1) as pa_w,
                tc.tile_pool(name="pa_x", bufs=1) as pa_x,
            ):
                wt = {}
                for w in "qkv":
                    for a in "hl":
                        wt[(w, a)] = pa_w.tile([P, NP8, 2, CQ], fp8,
                                               name=f"w{w}{a}",
                                               tag=f"w{w}{a}", bufs=1)
                for a in "hl":
                    nc.sync.dma_start(wt[("q", a)][:, 0:1],
                                      wd[("q", a)][:, 0:1])
                for hp, lo, hi in ((0, 1, 4), (1, 4, 8)):
                    for w in "qkv":
                        for a in "hl":
                            if w == "q":
                                nc.sync.dma_start(wt[(w, a)][:, lo:hi],
                                                  wd[(w, a)][:, lo:hi])
                            else:
                                nc.sync.dma_start(
                                    wt[(w, a)][:, hp * 4:(hp + 1) * 4],
                                    wd[(w, a)][:, hp * 4:(hp + 1) * 4])

                xe_blocks = [{}, {}]

                def get_xe(bo, pair, a):
                    # tags are shared between the two t-blocks (16 tags,
                    # bufs=1): block 1's DMA starts as soon as block 0's
                    # last reader of that tag is done
                    xe = xe_blocks[bo]
                    if (pair, a) not in xe:
                        tl = pa_x.tile([P, 2, 1024], fp8,
                                       name=f"xe{bo}{pair}{a}",
                                       tag=f"xe{pair}{a}", bufs=1)
                        nc.gpsimd.dma_start(
                            tl, xd[a][pair, :, :, bo * 1024:(bo + 1) * 1024])
                        xe[(pair, a)] = tl
                    return xe[(pair, a)]

                # A-round psum tiles borrow phase B's tag slots (2x[P,1024]
                # "psa" + pso0 + pso1 + 2x[P,512] "pqs" = exactly 8 banks);
                # the returned list holds 8 [P,512] views indexed c*2+half.
                def alloc_round_psums():
                    pa0 = pb_ps.tile([P, 1024], f32, name="pssA",
                                     tag="psa", bufs=2)
                    pa1 = pb_ps.tile([P, 1024], f32, name="pssB",
                                     tag="psa", bufs=2)
                    o0 = pb_ps.tile([P, 512], f32, name="pssC",
                                    tag="pso0", bufs=1)
                    o1 = pb_ps.tile([P, 512], f32, name="pssD",
                                    tag="pso1", bufs=1)
                    q0 = pb_ps.tile([P, 512], f32, name="pssE",
                                    tag="pqs", bufs=2)
                    q1 = pb_ps.tile([P, 512], f32, name="pssF",
                                    tag="pqs", bufs=2)
                    return [pa0[:, 0:512], pa0[:, 512:1024],
                            pa1[:, 0:512], pa1[:, 512:1024],
                            o0, o1, q0, q1]

                if True:
                    # --- A prefix: b0 [Q K V], b1 [K V] ---
                    def emit_qk_round(bo, w, dst):
                        t0 = bo * 1024
                        pss = alloc_round_psums()
                        def qk_mm(pair, ti, c, half):
                            xa, wb = TERMS[ti]
                            nc.tensor.matmul(
                                pss[c * 2 + half],
                                lhsT=wt[(w, wb)][:, pair, :,
                                                 c * P:(c + 1) * P],
                                rhs=get_xe(bo, pair, xa)[
                                    :, :, half * 512:(half + 1) * 512],
                                perf_mode=DR,
                                start=(pair == 0 and ti == 0),
                                stop=(pair == NP8 - 1 and ti == 2),
                            )

                        for pair in range(NP8 - 1):
                            for ti in range(3):
                                for c in range(4):
                                    for half in range(2):
                                        qk_mm(pair, ti, c, half)
                        # last pair: finish view (c,half) completely before
                        # moving on, so its psum copy overlaps the round
                        # tail instead of stalling the next round
                        for c in range(4):
                            for half in range(2):
                                for ti in range(3):
                                    qk_mm(NP8 - 1, ti, c, half)
                        for c in range(4):
                            for half in range(2):
                                d_ = dst[:, c,
                                         t0 + half * 512:t0 + (half + 1) * 512]
                                if (c * 2 + half) % 2 == 0:
                                    nc.scalar.copy(d_, pss[c * 2 + half])
                                else:
                                    nc.vector.tensor_copy(d_, pss[c * 2 + half])

                    def emit_v_round(bo):
                        psv = alloc_round_psums()
                        def v_mm(pair, ti, tj):
                            xa, wb = TERMS[ti]
                            nc.tensor.matmul(
                                psv[tj],
                                lhsT=get_xe(bo, pair, xa)[
                                    :, :, tj * P:(tj + 1) * P],
                                rhs=wt[("v", wb)][:, pair, :, :],
                                perf_mode=DR,
                                start=(pair == 0 and ti == 0),
                                stop=(pair == NP8 - 1 and ti == 2),
                            )

                        for pair in range(NP8 - 1):
                            for ti in range(3):
                                for tj in range(8):
                                    v_mm(pair, ti, tj)
                        for tj in range(8):
                            for ti in range(3):
                                v_mm(NP8 - 1, ti, tj)
                        # split the copies of the last A round between
                        # DVE and ACT so the trailing drain that gates
                        # phase B's first PSUM allocations halves
                        for tj in range(8):
                            dst = vsb[:, bo * 8 + tj, :]
                            if tj % 2 == 0:
                                nc.scalar.copy(dst, psv[tj])
                            else:
                                nc.vector.tensor_copy(dst, psv[tj])

                    emit_qk_round(0, "q", qt)
                    emit_qk_round(0, "k", kt)
                    emit_v_round(0)
                    emit_qk_round(1, "k", kt)
                    emit_v_round(1)

                # --- B(tb0) with the b1-Q round injected 3 mm per s ---
                # sub-round h covers qt chunk c==h (heads line up with the
                # consumer B(tb1, h)); s-iters 0-7 accumulate half 0,
                # s-iters 8-15 half 1.
                for h in range(NHC):
                    state = {}

                    def q_hook(s, h=h, state=state):
                        half = s // 8
                        j = (s % 8) * 3           # 24 mms per half
                        if j == 0:
                            state[half] = pb_ps.tile(
                                [P, 512], f32, name="pqs", tag="pqs",
                                bufs=2)
                        pq = state[half]
                        for k in range(3):
                            idx = j + k
                            pair, ti = divmod(idx, 3)
                            xa, wb = TERMS[ti]
                            xt = get_xe(1, pair, xa)
                            nc.tensor.matmul(
                                pq,
                                lhsT=wt[("q", wb)][:, pair, :,
                                                   h * P:(h + 1) * P],
                                rhs=xt[:, :, half * 512:(half + 1) * 512],
                                perf_mode=DR,
                                start=(idx == 0), stop=(idx == 23),
                            )
                        if s % 8 == 7:
                            nc.vector.tensor_copy(
                                qt[:, h,
                                   1024 + half * 512:1024 + (half + 1) * 512],
                                pq)

                    emit_B(0, h, s_hook=q_hook)

            with tc.tile_pool(name="pd", bufs=1) as pd_pool:
                for h in range(NHC):
                    emit_B(1, h)
                    emit_D(0, 2 * h, pd_pool)
                    emit_D(0, 2 * h + 1, pd_pool)
                emit_D1_all(pd_pool)

    nc.compile()
    return nc


def _get_nc():
    global _NC_CACHE
    if _NC_CACHE is None:
        _NC_CACHE = _build_nc()
    return _NC_CACHE


def _split8(a, s):
    """a*s ~= hi + lo, both e4m3 at a common scale; returns fp8 arrays."""
    import ml_dtypes
    f8 = ml_dtypes.float8_e4m3fn
    a = np.asarray(a, np.float32) * s
    amax = float(np.abs(a).max())
    assert amax < 200.0, f"fp8 range: scaled absmax {amax}"
    hi = a.astype(f8)
    lo = (a - hi.astype(np.float32)).astype(f8)
    return hi, lo


def _interleave(a):
    """[E, M] -> [NP8, 128, 2, M] with e = pair*256 + i*128 + p."""
    M = a.shape[1]
    return np.ascontiguousarray(
        a.reshape(NP8, 2, P, M).transpose(0, 2, 1, 3))


def _interleave_w(a):
    """[E, M] -> [128, NP8, 2, M] with e = pair*256 + i*128 + p."""
    M = a.shape[1]
    return np.ascontiguousarray(
        a.reshape(NP8, 2, P, M).transpose(2, 0, 1, 3))


def _shard_inputs(x, W_attn, W_proj, lambda_q1, lambda_k1,
                  lambda_q2, lambda_k2):
    import ml_dtypes
    bf16 = ml_dtypes.bfloat16
    x = np.asarray(x, np.float32)
    W_attn = np.asarray(W_attn, np.float32)
    W_proj = np.asarray(W_proj, np.float32)
    lam = float(np.exp(np.dot(np.asarray(lambda_q1, np.float32),
                              np.asarray(lambda_k1, np.float32)))
                - np.exp(np.dot(np.asarray(lambda_q2, np.float32),
                                np.asarray(lambda_k2, np.float32)))
                + LAMBDA_INIT)
    Cb = E // 2  # 1024: q1/k1/q2/k2 block width in W_attn

    # x splits are shared by the 4 cores of each batch element
    xsplit = []
    for b in range(B):
        xh, xl = _split8(np.ascontiguousarray(x[b].T), SX)
        xsplit.append((_interleave(xh), _interleave(xl)))

    in_maps = []
    for c in range(N_CORES):
        b, hg = divmod(c, 4)
        heads = [4 * hg + j for j in range(NHC)]
        wq_c = np.empty((E, CQ), np.float32)
        wk_c = np.empty((E, CQ), np.float32)
        wv_c = np.empty((E, CQ), np.float32)
        wp_c = np.empty((CQ, E), np.float32)
        for j, h in enumerate(heads):
            wq_c[:, j * P:j * P + HD] = W_attn[:, h * HD:(h + 1) * HD] * SCALE
            wq_c[:, j * P + HD:(j + 1) * P] = (
                W_attn[:, 2 * Cb + h * HD:2 * Cb + (h + 1) * HD]
                * (-lam * SCALE))
            wk_c[:, j * P:j * P + HD] = W_attn[:, Cb + h * HD:Cb + (h + 1) * HD]
            wk_c[:, j * P + HD:(j + 1) * P] = (
                W_attn[:, 3 * Cb + h * HD:3 * Cb + (h + 1) * HD])
            wv_c[:, j * P:(j + 1) * P] = (
                W_attn[:, 4 * Cb + h * DV:4 * Cb + (h + 1) * DV])
            wp_c[j * P:(j + 1) * P, :] = (
                W_proj[h * DV:(h + 1) * DV, :]
                * ((1.0 - LAMBDA_INIT) / (SX * SWV)))
        wqh, wql = _split8(wq_c, SWQ)
        wkh, wkl = _split8(wk_c, SWK)
        wvh, wvl = _split8(wv_c, SWV)
        wph, wpl = _split8(wp_c, SWP)

        def _ilv_p(a):
            # [CQ, E] -> [P, 2jp, 2i, E] with dv = jp*256 + i*128 + p
            return np.ascontiguousarray(
                a.reshape(2, 2, P, E).transpose(2, 0, 1, 3))

        xh8, xl8 = xsplit[b]
        in_maps.append({
            "xh": xh8, "xl": xl8,
            "wqh": _interleave_w(wqh), "wql": _interleave_w(wql),
            "wkh": _interleave_w(wkh), "wkl": _interleave_w(wkl),
            "wvh": _interleave_w(wvh), "wvl": _interleave_w(wvl),
            "wph": _ilv_p(wph), "wpl": _ilv_p(wpl),
        })
    return in_maps


def _run(inputs, trace=False):
    from concourse.bass_utils import run_bass_kernel_spmd
    nc = _get_nc()
    in_maps = _shard_inputs(**inputs)
    res = run_bass_kernel_spmd(nc, in_maps, list(range(N_CORES)),
                               trace=trace)
    out = np.zeros((B, T, E), np.float32)
    for c in range(N_CORES):
        out[c // 4] += res.results[c]["out"].astype(np.float32)
    out *= S_OUT
    return out, res


def kernel(x, W_attn, W_proj, lambda_q1, lambda_k1, lambda_q2, lambda_k2):
    out, _ = _run(dict(x=x, W_attn=W_attn, W_proj=W_proj,
                       lambda_q1=lambda_q1, lambda_k1=lambda_k1,
                       lambda_q2=lambda_q2, lambda_k2=lambda_k2))
    return out
